# revision 29
# baseline (speedup 1.0000x reference)
"""Bass/Trainium2 kernel for BiGraphContrastLayer (GNN message passing).

Computes, for two edge lists (pos/neg) over the same node features:
    h_g = PReLU( D_in^-1/2 A_g D_out^-1/2 feats @ W + b )
returning stack([h_pos, h_neg]) of shape [2, N, Dout].

Strategy (8 NeuronCores, SPMD, no collectives), using the linearity
   (D_in^-1/2 A D_out^-1/2 feats) @ W = (D_in^-1/2 A D_out^-1/2 feats) W:

  No y-phase: dma_gather (int16 idx, 4 src-row banks of <=25088 rows)
  pulls RAW bf16 feats rows straight from the (host-cast, padded) input.
  Edges are bucketed by dst tile (slot), dealt to cores, and packed
  CONTINUOUSLY per (supergroup, bank) into 128-row chunks — a chunk may
  span several dst slots; per (chunk, slot) a weighted one-hot
  (rhs[p, j] = ns[src_p]*nd[dst_p] iff slot-relative dst position match,
  built on DVE via is_equal(int16 ramp slice, f32 off) + mult) matmul
  accumulates into the slot's quarter of a quad [128, 512] PSUM bank
  aggT[feat, dst]. Per quad: one cast copy aggT -> SBUF bf16, per slot a
  matmul (lhsT=aggT slice, rhs=W) -> h PSUM quad, one PReLU on ScalarE,
  bf16 store in a p-major layout (contiguous per partition).

  All 8 cores share one instruction stream: the chunk/matmul schedule is
  the UNION over cores (max chunk counts, union slot spans); cores
  lacking edges for a scheduled (chunk, slot) build an all-zero one-hot.

  Host does index/metadata work only: degree bincounts -> per-edge norm
  weights, sorting, bucketing, dealing, packing, int16 wrapped gather
  indices, replicating the small W/b/prelu params per the sharding hint.
"""

import math
import tempfile
from dataclasses import dataclass

import numpy as np

P = 128   # partitions
D = 128   # feature dim (Din == Dout == 128)
NBANK = 4
BF16 = np.dtype("bfloat16")
OFF_PAD = 4096.0  # off value matching no ramp slice


# --------------------------------------------------------------------------
# Config
# --------------------------------------------------------------------------
@dataclass
class Config:
    n_nodes: int = 100000
    n_cores: int = 8
    sg: int = 12       # dst-tile slots per supergroup (multiple of 4 best)
    oh_gpsimd_mod: int = 0    # every Nth one-hot build goes to GpSimd (0=off)
    act_prelu: bool = True    # final PReLU on ScalarE (not in sim)
    gbufs: int = 3            # gather buffer count
    ipbufs: int = 3           # idx buffer count
    ppbufs: int = 6           # PSUM quad accumulator banks
    hpbufs: int = 2           # PSUM h banks

    @property
    def t_global(self) -> int:
        return math.ceil(self.n_nodes / P)

    @property
    def n_pad(self) -> int:
        return self.t_global * P

    @property
    def t_core(self) -> int:
        return math.ceil(self.t_global / self.n_cores)

    @property
    def bank_tiles(self) -> int:
        return math.ceil(self.t_global / NBANK)

    @property
    def bank_rows(self) -> int:
        return self.bank_tiles * P

    @property
    def n_sg(self) -> int:
        return math.ceil(self.t_core / self.sg)


# --------------------------------------------------------------------------
# Host-side preprocessing (integer index / edge-weight metadata only)
# --------------------------------------------------------------------------
def _plan_graph(src, dst, nse_edge, cfg: Config):
    """Bucket edges by dst tile, sort by src within tile, bank-split, and
    deal tiles to cores (snake by total edge count for balance).

    Returns dict with:
      core_tiles  [n_cores, t_core]  global tile id per slot (-1 null)
      counts      [n_cores, t_core, NBANK] per-slot-bank edge counts
      tile_edges  list per global tile, per bank: (src_local, dstoff, nse)
    """
    tg, ncores, tcore = cfg.t_global, cfg.n_cores, cfg.t_core
    order = np.argsort(dst, kind="stable")
    src_s = src[order]
    dst_s = dst[order]
    nse_s = nse_edge[order]
    tile_cnt = np.bincount(dst_s // P, minlength=tg)
    starts = np.zeros(tg + 1, np.int64)
    np.cumsum(tile_cnt, out=starts[1:])

    bank_of = src_s // cfg.bank_rows
    tile_edges = []
    for t in range(tg):
        e0, e1 = int(starts[t]), int(starts[t + 1])
        per_bank = []
        for b in range(NBANK):
            m = bank_of[e0:e1] == b
            per_bank.append((
                (src_s[e0:e1][m] - b * cfg.bank_rows).astype(np.int64),
                (dst_s[e0:e1][m] % P).astype(np.int64),
                nse_s[e0:e1][m].astype(np.float32),
            ))
        tile_edges.append(per_bank)

    # Deal tiles by descending total edges; within each rank-group of
    # ncores tiles, greedily give each tile to the core whose running
    # per-bank supergroup sums stay smallest (bank-aware LPT) — the
    # shared per-(sg, bank) chunk count is the max over cores, so
    # minimizing the max per-bank deficit minimizes gather padding.
    bank_cnt = np.zeros((tg, NBANK), np.int64)
    for t in range(tg):
        for b in range(NBANK):
            bank_cnt[t, b] = len(tile_edges[t][b][0])
    keys = np.argsort(tile_cnt, kind="stable")[::-1]
    core_tiles = np.full((ncores, tcore), -1, np.int64)
    run = np.zeros((ncores, NBANK), np.int64)
    for k in range(tcore):
        if k % cfg.sg == 0:
            run[:] = 0  # new supergroup
        grp = keys[k * ncores : (k + 1) * ncores]
        free = list(range(ncores))
        for t in grp:
            proj = run[free] + bank_cnt[t]
            i = int(np.argmin(proj.max(axis=1) + 1e-3 * proj.sum(axis=1)))
            c = free.pop(i)
            core_tiles[c, k] = t
            run[c] += bank_cnt[t]

    counts = np.zeros((ncores, tcore, NBANK), np.int64)
    for c in range(ncores):
        for k in range(tcore):
            t = core_tiles[c, k]
            if t < 0:
                continue
            for b in range(NBANK):
                counts[c, k, b] = len(tile_edges[t][b][0])
    return dict(core_tiles=core_tiles, counts=counts, tile_edges=tile_edges)


def _schedule_graph(plan, cfg: Config):
    """Shared (all-cores) chunk layout + matmul schedule for one graph.

    Chunks are packed continuously per (supergroup, bank): each core lays
    its slots' bank-b edges end-to-end; the shared chunk count is the max
    over cores, the per-chunk slot list is the union over cores.

    Returns dict:
      sg_list  [(k0, kn)]
      nch      [n_sg][NBANK] shared chunk counts
      c0       [n_sg][NBANK] global first-chunk index
      mms      [n_sg] ordered list of (bank, j, slot_local, start, stop)
      n_chunks total
    """
    counts = plan["counts"]
    ncores, tcore = cfg.n_cores, cfg.t_core
    # Supergroups: full-size bodies, but split the tail into small sgs so
    # the end-of-kernel drain (matmul/copy/W/prelu/store of the last sg)
    # is short and overlaps the final gathers.
    sg_list = []
    k0 = 0
    while k0 < tcore:
        rem = tcore - k0
        if rem > cfg.sg + cfg.sg // 2:
            kn = cfg.sg
        elif rem > cfg.sg:
            kn = (rem + 1) // 2  # two medium tail sgs
        elif rem > 4:
            kn = (rem + 1) // 2  # two small tail sgs
        else:
            kn = rem
        sg_list.append((k0, kn))
        k0 += kn

    nch_all, c0_all, mms_all, spans_all, nidx_all = [], [], [], [], []
    cpos = 0
    for (k0, kn) in sg_list:
        nch_sg = []
        c0_sg = []
        nidx_sg = []
        touches = []  # (slot, bank, j) -> sorted slot-major for PSUM groups
        covered = set()
        for b in range(NBANK):
            cum = np.zeros((ncores, kn + 1), np.int64)
            np.cumsum(counts[:, k0 : k0 + kn, b], axis=1, out=cum[:, 1:])
            maxn = int(max(cum[c, kn] for c in range(ncores)))
            nidx = -(-maxn // 16) * 16  # descriptors charged = num_idxs
            nch = -(-nidx // P)
            nch_sg.append(nch)
            nidx_sg.append(nidx)
            c0_sg.append(cpos)
            cpos += nch
            for j in range(nch):
                lo, hi = j * P, (j + 1) * P
                slots = set()
                for c in range(ncores):
                    for ki in range(kn):
                        if cum[c, ki] < hi and cum[c, ki + 1] > lo:
                            slots.add(ki)
                for ki in sorted(slots):
                    touches.append((ki, b, j))
                    covered.add(ki)
        # slot-major order: each slot's PSUM accumulation group closes
        # before the next one opens in the same PSUM bank
        entries = [(b, j, ki) for (ki, b, j) in sorted(touches)]
        # per-chunk slot span (for the wide one-hot build)
        spans = {}
        for (ki, b, j) in touches:
            lo, hi = spans.get((b, j), (ki, ki))
            spans[(b, j)] = (min(lo, ki), max(hi, ki))
        if sum(nch_sg) == 0:
            # fully empty supergroup: force one pad chunk in bank 0
            nch_sg[0] = 1
            nidx_sg[0] = 16
            for b in range(1, NBANK):
                c0_sg[b] = c0_sg[0] + 1
            cpos += 1
        # zero-coverage slots get one all-zero matmul on the sg's first
        # populated bank's chunk 0 (no core has a matching off there, by
        # construction)
        dummy_bank = next(b for b in range(NBANK) if nch_sg[b] > 0)
        for ki in range(kn):
            if ki not in covered:
                entries.append((dummy_bank, 0, ki))
        # start/stop per slot over entry order
        first, last = {}, {}
        for i, (b, j, ki) in enumerate(entries):
            if ki not in first:
                first[ki] = i
            last[ki] = i
        mms = [(b, j, ki, i == first[ki], i == last[ki])
               for i, (b, j, ki) in enumerate(entries)]
        nch_all.append(nch_sg)
        c0_all.append(c0_sg)
        mms_all.append(mms)
        spans_all.append(spans)
        nidx_all.append(nidx_sg)
    # idx column layout: per (sg, bank) call, nidx/16 int16 columns
    icol_all = []
    icol = 0
    for si in range(len(sg_list)):
        icol_sg = []
        for b in range(NBANK):
            icol_sg.append(icol)
            icol += nidx_all[si][b] // 16
        icol_all.append(icol_sg)
    return dict(sg_list=sg_list, nch=nch_all, c0=c0_all, mms=mms_all,
                spans=spans_all, nidx=nidx_all, icol=icol_all,
                n_chunks=cpos, ncols=icol)


def _fill_core_graph(plan, sched, core, cfg: Config):
    """Build IDX16 (wrapped), OFF and NSE arrays for one core, one graph."""
    n_chunks = sched["n_chunks"]
    idx = np.zeros((n_chunks, P), np.int16)
    off = np.full((n_chunks, P), OFF_PAD, np.float32)
    nse = np.zeros((n_chunks, P), np.float32)
    core_tiles = plan["core_tiles"]
    tile_edges = plan["tile_edges"]
    for si, (k0, kn) in enumerate(sched["sg_list"]):
        for b in range(NBANK):
            nch = sched["nch"][si][b]
            if nch == 0:
                continue
            c0 = sched["c0"][si][b]
            es = np.zeros(nch * P, np.int64)
            eo = np.full(nch * P, OFF_PAD, np.float32)
            en = np.zeros(nch * P, np.float32)
            pos = 0
            for ki in range(kn):
                t = core_tiles[core, k0 + ki]
                if t < 0:
                    continue
                s_rows, s_off, s_nse = tile_edges[t][b]
                nb = len(s_rows)
                es[pos : pos + nb] = s_rows
                eo[pos : pos + nb] = ki * P + s_off
                en[pos : pos + nb] = s_nse
                pos += nb
            idx[c0 : c0 + nch] = es.reshape(nch, P)
            off[c0 : c0 + nch] = eo.reshape(nch, P)
            nse[c0 : c0 + nch] = en.reshape(nch, P)
    # wrap: flat slot i (within a call's first num_idxs slots) ->
    # [i%16, i//16], replicated to 128 partitions. Only num_idxs (<=
    # nch*128, 16-aligned) indices are shipped/charged per call; the
    # remaining tail of the last chunk is never gathered and its off
    # stays OFF_PAD.
    idx_w = np.zeros((P, sched["ncols"]), np.int16)
    for si in range(len(sched["sg_list"])):
        for b in range(NBANK):
            nidx = sched["nidx"][si][b]
            if nidx == 0:
                continue
            c0 = sched["c0"][si][b]
            icol = sched["icol"][si][b]
            flat = idx[c0 : c0 + sched["nch"][si][b]].reshape(-1)[:nidx]
            blk = flat.reshape(-1, 16).T  # [16, nidx/16]
            idx_w[:, icol : icol + nidx // 16] = np.tile(blk, (8, 1))
    # interleave off/nse per supergroup block ([off_cols | nse_cols]) so
    # the per-sg DMA load is one >=512B-per-partition transfer (no 2x
    # small-descriptor penalty)
    offT, nseT = off.T, nse.T
    onse = np.empty((P, 2 * n_chunks), np.float32)
    for si in range(len(sched["sg_list"])):
        nchs = sched["nch"][si]
        c0_sg = min(sched["c0"][si][b] for b in range(NBANK) if nchs[b] > 0)
        sgc = sum(nchs)
        onse[:, 2 * c0_sg : 2 * c0_sg + sgc] = offT[:, c0_sg : c0_sg + sgc]
        onse[:, 2 * c0_sg + sgc : 2 * (c0_sg + sgc)] = (
            nseT[:, c0_sg : c0_sg + sgc])
    return idx_w, onse  # [P, 2*n_chunks]


def preprocess(feats, W, b, prelu_a, src_pos, dst_pos, src_neg, dst_neg,
               cfg: Config):
    n, ncores = cfg.n_nodes, cfg.n_cores
    feats = np.asarray(feats, np.float32)
    W = np.asarray(W, np.float32)
    b = np.asarray(b, np.float32)
    prelu_a = np.asarray(prelu_a, np.float32)

    featsr = np.zeros((cfg.n_pad, D), BF16)  # row-major, padded, bf16
    featsr[:n] = feats.astype(BF16)

    plans, scheds = [], []
    for src, dst in ((src_pos, dst_pos), (src_neg, dst_neg)):
        src = np.asarray(src, np.int64)
        dst = np.asarray(dst, np.int64)
        deg_out = np.bincount(src, minlength=n).astype(np.float32)
        deg_in = np.bincount(dst, minlength=n).astype(np.float32)
        ns = np.where(deg_out > 0, 1.0 / np.sqrt(np.maximum(deg_out, 1.0)),
                      0.0).astype(np.float32)
        nd = np.where(deg_in > 0, 1.0 / np.sqrt(np.maximum(deg_in, 1.0)),
                      0.0).astype(np.float32)
        nse_edge = ns[src] * nd[dst]
        plan = _plan_graph(src, dst, nse_edge, cfg)
        plans.append(plan)
        scheds.append(_schedule_graph(plan, cfg))

    ramp = np.tile(np.arange(cfg.sg * P, dtype=np.int16), (P, 1))
    a_rep = np.full((P, 1), float(prelu_a.reshape(-1)[0]), np.float32)
    b_rep = np.tile(b.reshape(1, D), (P, 4)).astype(np.float32)

    in_maps = []
    for core in range(ncores):
        iw_p, onse_p = _fill_core_graph(plans[0], scheds[0], core, cfg)
        iw_n, onse_n = _fill_core_graph(plans[1], scheds[1], core, cfg)
        in_maps.append({
            "featsr": featsr,
            "w_in": W,
            "a_rep": a_rep,
            "b_rep": b_rep,
            "idx_in": np.concatenate([iw_p, iw_n], axis=1),
            "onse_in": np.concatenate([onse_p, onse_n], axis=1),
            "ramp_in": ramp,
        })
    meta = {
        "scheds": scheds,
        "use_bias": bool(np.any(b != 0.0)),
    }
    return in_maps, plans, meta


# --------------------------------------------------------------------------
# Device kernel builder
# --------------------------------------------------------------------------
def build_kernel(nc, tc, cfg: Config, meta):
    from contextlib import ExitStack

    import concourse.mybir as mybir

    f32 = mybir.dt.float32
    bf16 = mybir.dt.bfloat16
    i16 = mybir.dt.int16
    Alu = mybir.AluOpType
    Act = mybir.ActivationFunctionType

    npad = cfg.n_pad
    scheds = meta["scheds"]
    use_bias = meta["use_bias"]
    n_chunks = [scheds[g]["n_chunks"] for g in range(2)]
    ncols = [scheds[g]["ncols"] for g in range(2)]
    n_sg = len(scheds[0]["sg_list"])
    assert len(scheds[1]["sg_list"]) == n_sg

    featsr = nc.dram_tensor("featsr", [npad, D], bf16,
                            kind="ExternalInput").ap()
    w_in = nc.dram_tensor("w_in", [P, D], f32, kind="ExternalInput").ap()
    a_rep = nc.dram_tensor("a_rep", [P, 1], f32, kind="ExternalInput").ap()
    b_rep = nc.dram_tensor("b_rep", [P, 4 * D], f32, kind="ExternalInput").ap()
    idx_in = nc.dram_tensor("idx_in", [P, sum(ncols)], i16,
                            kind="ExternalInput").ap()
    onse_in = nc.dram_tensor("onse_in", [P, 2 * sum(n_chunks)], f32,
                             kind="ExternalInput").ap()
    ramp_in = nc.dram_tensor("ramp_in", [P, cfg.sg * P], i16,
                             kind="ExternalInput").ap()
    out = nc.dram_tensor("out", [2, n_sg, P, cfg.sg * D], bf16,
                         kind="ExternalOutput").ap()

    with ExitStack() as ctx:
        const = ctx.enter_context(tc.tile_pool(name="const", bufs=1))
        gpool = ctx.enter_context(tc.tile_pool(name="gpool", bufs=cfg.gbufs))
        ipool = ctx.enter_context(tc.tile_pool(name="ipool", bufs=cfg.ipbufs))
        ohpool = ctx.enter_context(tc.tile_pool(name="ohpool", bufs=24))
        aggpool = ctx.enter_context(tc.tile_pool(name="aggpool", bufs=4))
        tpool = ctx.enter_context(tc.tile_pool(name="tpool", bufs=4))
        spool = ctx.enter_context(tc.tile_pool(name="spool", bufs=3))
        ppool = ctx.enter_context(tc.tile_pool(name="ppool", bufs=cfg.ppbufs,
                                               space="PSUM"))
        hpool = ctx.enter_context(tc.tile_pool(name="hpool", bufs=cfg.hpbufs,
                                               space="PSUM"))

        # ---- constants ----
        w_sb = const.tile([P, D], bf16)
        nc.gpsimd.dma_start(out=w_sb[:], in_=w_in)  # f32 -> bf16 cast DMA
        ramp_sb = const.tile([P, cfg.sg * P], i16)
        nc.sync.dma_start(out=ramp_sb[:], in_=ramp_in)
        a_sb = const.tile([P, 1], f32)
        nc.sync.dma_start(out=a_sb[:], in_=a_rep)
        if use_bias:
            b_sb = const.tile([P, 4 * D], f32)
            nc.sync.dma_start(out=b_sb[:], in_=b_rep)

        max_sgc = max(sum(scheds[g]["nch"][si]) for g in range(2)
                      for si in range(n_sg))

        # ---- gather + weighted one-hot segment-sum + @W + prelu ----
        col_base = [0, ncols[0]]          # idx column offset per graph
        chk_base = [0, 2 * n_chunks[0]]   # onse column offset per graph
        # interleave the two graphs' supergroups so one graph's gathers fill
        # DMA while the other's PSUM chain drains
        jobs = []
        for si in range(n_sg):
            for g in range(2):
                jobs.append((g, si))
        ecnt = 0
        for (g, si) in jobs:
            sch = scheds[g]
            (k0, kn) = sch["sg_list"][si]
            nchs = sch["nch"][si]
            c0s = sch["c0"][si]
            c0_sg = min(c0s[b] for b in range(NBANK) if nchs[b] > 0)
            sg_chunks = sum(nchs)
            nidxs = sch["nidx"][si]
            icols = sch["icol"][si]
            icol_sg = icols[0]
            icol_w = sum(nidxs) // 16
            gt = gpool.tile([P, max_sgc, D], bf16, tag="gather")
            it = ipool.tile([P, icol_w], i16, tag="gidx")
            nc.sync.dma_start(
                out=it[:],
                in_=idx_in[:, col_base[g] + icol_sg :
                           col_base[g] + icol_sg + icol_w])
            oet = ipool.tile([P, 2 * sg_chunks], f32, tag="gonse")
            nc.sync.dma_start(
                out=oet[:],
                in_=onse_in[:, chk_base[g] + 2 * c0_sg :
                            chk_base[g] + 2 * (c0_sg + sg_chunks)])
            ot = oet[:, :sg_chunks]
            et = oet[:, sg_chunks:]
            for b in range(NBANK):
                nch = nchs[b]
                if nch == 0:
                    continue
                lo = c0s[b] - c0_sg
                ilo = icols[b] - icol_sg
                nidx = nidxs[b]
                bank_rows = min(cfg.bank_rows, npad - b * cfg.bank_rows)
                nc.gpsimd.dma_gather(
                    out_ap=gt[:, lo : lo + nch, :],
                    in_ap=featsr[b * cfg.bank_rows :
                                 b * cfg.bank_rows + bank_rows, :],
                    idxs_ap=it[:, ilo : ilo + nidx // 16],
                    num_idxs=nidx, num_idxs_reg=nidx,
                    elem_size=D, single_packet=False)
            nquad = (kn + 3) // 4
            psTs = [ppool.tile([P, 4 * D], f32, tag="psT", name="psT")
                    for _ in range(nquad)]
            spans = sch["spans"][si]
            oh_cache = {}
            for (b, j, ki, st, sp) in sch["mms"][si]:
                lo = c0s[b] - c0_sg + j
                pv = min(P, nidxs[b] - j * P)  # valid rows in this chunk
                klo, khi = spans.get((b, j), (ki, ki))
                if ki < klo or ki > khi:
                    # dummy zero matmul for an uncovered slot: one-off
                    # narrow one-hot (off values there never match ki)
                    oh = ohpool.tile([P, D], bf16, tag="ohw", name="ohw")
                    nc.vector.tensor_scalar(
                        out=oh[:], in0=ramp_sb[:, ki * P : (ki + 1) * P],
                        scalar1=ot[:, lo : lo + 1],
                        scalar2=et[:, lo : lo + 1],
                        op0=Alu.is_equal, op1=Alu.mult)
                    rhs = oh[:pv, :]
                else:
                    if (b, j) not in oh_cache:
                        span = khi - klo + 1
                        ohw = ohpool.tile([P, span * D], bf16, tag="ohw",
                                          name="ohw")
                        eng = nc.vector
                        if cfg.oh_gpsimd_mod and (
                                ecnt % cfg.oh_gpsimd_mod == 0):
                            eng = nc.gpsimd
                        ecnt += 1
                        eng.tensor_scalar(
                            out=ohw[:],
                            in0=ramp_sb[:, klo * P : (khi + 1) * P],
                            scalar1=ot[:, lo : lo + 1],
                            scalar2=et[:, lo : lo + 1],
                            op0=Alu.is_equal, op1=Alu.mult)
                        oh_cache[(b, j)] = ohw
                    ohw = oh_cache[(b, j)]
                    rhs = ohw[:pv, (ki - klo) * D : (ki - klo + 1) * D]
                q, r = divmod(ki, 4)
                nc.tensor.matmul(
                    out=psTs[q][:, r * D : (r + 1) * D],
                    lhsT=gt[:pv, lo, :], rhs=rhs, start=st, stop=sp)
            stg = spool.tile([P, kn * D], bf16, tag="stg")
            for q in range(nquad):
                kq = min(4, kn - 4 * q)
                aggsb = aggpool.tile([P, 4 * D], bf16, tag="aggsb")
                nc.scalar.activation(out=aggsb[:, : kq * D],
                                     in_=psTs[q][:, : kq * D],
                                     func=Act.Copy)
                hps = hpool.tile([P, 4 * D], f32)
                for r in range(kq):
                    nc.tensor.matmul(out=hps[:, r * D : (r + 1) * D],
                                     lhsT=aggsb[:, r * D : (r + 1) * D],
                                     rhs=w_sb[:], start=True, stop=True)
                ss = stg[:, 4 * q * D : (4 * q + kq) * D]
                if use_bias:
                    hb2 = tpool.tile([P, 4 * D], f32, tag="hb2")
                    nc.vector.tensor_tensor(out=hb2[:, : kq * D],
                                            in0=hps[:, : kq * D],
                                            in1=b_sb[:, : kq * D], op=Alu.add)
                    neg = tpool.tile([P, 4 * D], f32, tag="neg")
                    nc.vector.tensor_scalar(
                        out=neg[:, : kq * D], in0=hb2[:, : kq * D],
                        scalar1=0.0, scalar2=a_sb[:, :1],
                        op0=Alu.min, op1=Alu.mult)
                    pos = tpool.tile([P, 4 * D], f32, tag="pos")
                    nc.vector.tensor_scalar(
                        out=pos[:, : kq * D], in0=hb2[:, : kq * D],
                        scalar1=0.0, scalar2=None, op0=Alu.max)
                    nc.vector.tensor_tensor(out=ss, in0=neg[:, : kq * D],
                                            in1=pos[:, : kq * D], op=Alu.add)
                elif cfg.act_prelu:
                    nc.scalar.activation(
                        out=ss, in_=hps[:, : kq * D], func=Act.Prelu,
                        alpha=a_sb[:, :1])
                else:
                    neg = tpool.tile([P, 4 * D], f32, tag="neg")
                    nc.vector.tensor_scalar(
                        out=neg[:, : kq * D], in0=hps[:, : kq * D],
                        scalar1=0.0, scalar2=a_sb[:, :1],
                        op0=Alu.min, op1=Alu.mult)
                    pos = tpool.tile([P, 4 * D], f32, tag="pos")
                    nc.vector.tensor_scalar(
                        out=pos[:, : kq * D], in0=hps[:, : kq * D],
                        scalar1=0.0, scalar2=None, op0=Alu.max)
                    nc.vector.tensor_tensor(out=ss, in0=neg[:, : kq * D],
                                            in1=pos[:, : kq * D], op=Alu.add)
            nc.sync.dma_start(out=out[g, si, :, : kn * D], in_=stg[:])
    return out


# --------------------------------------------------------------------------
# Driver
# --------------------------------------------------------------------------
def _build_program(cfg: Config, meta):
    import concourse.bacc as bacc
    import concourse.tile as tile

    nc = bacc.Bacc("TRN2", target_bir_lowering=False, debug=False,
                   enable_asserts=False, num_devices=cfg.n_cores)
    with tile.TileContext(nc) as tc:
        build_kernel(nc, tc, cfg, meta)
    nc.compile()
    return nc


def _unscramble(results, plans, meta, cfg: Config):
    n = cfg.n_nodes
    full = np.zeros((2, n, D), np.float32)
    for g in range(2):
        sched = meta["scheds"][g]
        ct_all = plans[g]["core_tiles"]
        for core in range(cfg.n_cores):
            oc = np.asarray(results[core]["out"], dtype=np.float32)
            for si, (k0, kn) in enumerate(sched["sg_list"]):
                for ki in range(kn):
                    t = int(ct_all[core, k0 + ki])
                    if t < 0:
                        continue
                    r0 = t * P
                    r1 = min(r0 + P, n)
                    full[g, r0:r1] = oc[g, si, : r1 - r0,
                                        ki * D : (ki + 1) * D]
    return full


_PROGRAM_CACHE = {}


def _sched_key(sched):
    return (tuple(tuple(x) for x in sched["nch"]),
            tuple(mm for sgm in sched["mms"] for mm in sgm))


def run(inputs, cfg: Config, trace=False):
    from concourse.bass_utils import run_bass_kernel_spmd

    in_maps, plans, meta = preprocess(
        inputs["feats"], inputs["W"], inputs["b"], inputs["prelu_a"],
        inputs["src_pos"], inputs["dst_pos"],
        inputs["src_neg"], inputs["dst_neg"], cfg)

    key = (cfg.n_nodes, cfg.n_cores, cfg.sg,
           cfg.act_prelu, cfg.oh_gpsimd_mod, cfg.gbufs,
           _sched_key(meta["scheds"][0]), _sched_key(meta["scheds"][1]),
           meta["use_bias"])
    nc = _PROGRAM_CACHE.get(key)
    if nc is None:
        nc = _build_program(cfg, meta)
        _PROGRAM_CACHE[key] = nc

    kwargs = {}
    if trace:
        kwargs = dict(trace=True, tmpdir=tempfile.mkdtemp(prefix="bgc_trace_"))
    res = run_bass_kernel_spmd(nc, in_maps, core_ids=list(range(cfg.n_cores)),
                               **kwargs)
    full = _unscramble(res.results, plans, meta, cfg)
    return full, res


def kernel(**inputs) -> np.ndarray:
    cfg = Config()
    full, _ = run(inputs, cfg)
    return full


# revision 30
# speedup vs baseline: 1.0107x; 1.0107x over previous
"""Bass/Trainium2 kernel for BiGraphContrastLayer (GNN message passing).

Computes, for two edge lists (pos/neg) over the same node features:
    h_g = PReLU( D_in^-1/2 A_g D_out^-1/2 feats @ W + b )
returning stack([h_pos, h_neg]) of shape [2, N, Dout].

Strategy (8 NeuronCores, SPMD, no collectives), using the linearity
   (D_in^-1/2 A D_out^-1/2 feats) @ W = (D_in^-1/2 A D_out^-1/2 feats) W:

  No y-phase: dma_gather (int16 idx, 4 src-row banks of <=25088 rows)
  pulls RAW bf16 feats rows straight from the (host-cast, padded) input.
  Edges are bucketed by dst tile (slot), dealt to cores, and packed
  CONTINUOUSLY per (supergroup, bank) into 128-row chunks — a chunk may
  span several dst slots; per (chunk, slot) a weighted one-hot
  (rhs[p, j] = ns[src_p]*nd[dst_p] iff slot-relative dst position match,
  built on DVE via is_equal(int16 ramp slice, f32 off) + mult) matmul
  accumulates into the slot's quarter of a quad [128, 512] PSUM bank
  aggT[feat, dst]. Per quad: one cast copy aggT -> SBUF bf16, per slot a
  matmul (lhsT=aggT slice, rhs=W) -> h PSUM quad, one PReLU on ScalarE,
  bf16 store in a p-major layout (contiguous per partition).

  All 8 cores share one instruction stream: the chunk/matmul schedule is
  the UNION over cores (max chunk counts, union slot spans); cores
  lacking edges for a scheduled (chunk, slot) build an all-zero one-hot.

  Host does index/metadata work only: degree bincounts -> per-edge norm
  weights, sorting, bucketing, dealing, packing, int16 wrapped gather
  indices, replicating the small W/b/prelu params per the sharding hint.
"""

import math
import tempfile
from dataclasses import dataclass

import numpy as np

P = 128   # partitions
D = 128   # feature dim (Din == Dout == 128)
NBANK = 4
BF16 = np.dtype("bfloat16")
OFF_PAD = 4096.0  # off value matching no ramp slice


# --------------------------------------------------------------------------
# Config
# --------------------------------------------------------------------------
@dataclass
class Config:
    n_nodes: int = 100000
    n_cores: int = 8
    sg: int = 16       # dst-tile slots per supergroup (multiple of 4 best)
    oh_gpsimd_mod: int = 0    # every Nth one-hot build goes to GpSimd (0=off)
    act_prelu: bool = True    # final PReLU on ScalarE (not in sim)
    gbufs: int = 3            # gather buffer count
    ipbufs: int = 4           # idx buffer count
    ppbufs: int = 6           # PSUM quad accumulator banks
    hpbufs: int = 2           # PSUM h banks

    @property
    def t_global(self) -> int:
        return math.ceil(self.n_nodes / P)

    @property
    def n_pad(self) -> int:
        return self.t_global * P

    @property
    def t_core(self) -> int:
        return math.ceil(self.t_global / self.n_cores)

    @property
    def bank_tiles(self) -> int:
        return math.ceil(self.t_global / NBANK)

    @property
    def bank_rows(self) -> int:
        return self.bank_tiles * P

    @property
    def n_sg(self) -> int:
        return math.ceil(self.t_core / self.sg)


# --------------------------------------------------------------------------
# Host-side preprocessing (integer index / edge-weight metadata only)
# --------------------------------------------------------------------------
def _plan_graph(src, dst, nse_edge, cfg: Config):
    """Bucket edges by dst tile, sort by src within tile, bank-split, and
    deal tiles to cores (snake by total edge count for balance).

    Returns dict with:
      core_tiles  [n_cores, t_core]  global tile id per slot (-1 null)
      counts      [n_cores, t_core, NBANK] per-slot-bank edge counts
      tile_edges  list per global tile, per bank: (src_local, dstoff, nse)
    """
    tg, ncores, tcore = cfg.t_global, cfg.n_cores, cfg.t_core
    order = np.argsort(dst, kind="stable")
    src_s = src[order]
    dst_s = dst[order]
    nse_s = nse_edge[order]
    tile_cnt = np.bincount(dst_s // P, minlength=tg)
    starts = np.zeros(tg + 1, np.int64)
    np.cumsum(tile_cnt, out=starts[1:])

    bank_of = src_s // cfg.bank_rows
    tile_edges = []
    for t in range(tg):
        e0, e1 = int(starts[t]), int(starts[t + 1])
        per_bank = []
        for b in range(NBANK):
            m = bank_of[e0:e1] == b
            per_bank.append((
                (src_s[e0:e1][m] - b * cfg.bank_rows).astype(np.int64),
                (dst_s[e0:e1][m] % P).astype(np.int64),
                nse_s[e0:e1][m].astype(np.float32),
            ))
        tile_edges.append(per_bank)

    # Deal tiles by descending total edges; within each rank-group of
    # ncores tiles, greedily give each tile to the core whose running
    # per-bank supergroup sums stay smallest (bank-aware LPT) — the
    # shared per-(sg, bank) chunk count is the max over cores, so
    # minimizing the max per-bank deficit minimizes gather padding.
    bank_cnt = np.zeros((tg, NBANK), np.int64)
    for t in range(tg):
        for b in range(NBANK):
            bank_cnt[t, b] = len(tile_edges[t][b][0])
    keys = np.argsort(tile_cnt, kind="stable")[::-1]
    core_tiles = np.full((ncores, tcore), -1, np.int64)
    run = np.zeros((ncores, NBANK), np.int64)
    for k in range(tcore):
        if k % cfg.sg == 0:
            run[:] = 0  # new supergroup
        grp = keys[k * ncores : (k + 1) * ncores]
        free = list(range(ncores))
        for t in grp:
            proj = run[free] + bank_cnt[t]
            i = int(np.argmin(proj.max(axis=1) + 1e-3 * proj.sum(axis=1)))
            c = free.pop(i)
            core_tiles[c, k] = t
            run[c] += bank_cnt[t]

    counts = np.zeros((ncores, tcore, NBANK), np.int64)
    for c in range(ncores):
        for k in range(tcore):
            t = core_tiles[c, k]
            if t < 0:
                continue
            for b in range(NBANK):
                counts[c, k, b] = len(tile_edges[t][b][0])
    return dict(core_tiles=core_tiles, counts=counts, tile_edges=tile_edges)


def _schedule_graph(plan, cfg: Config):
    """Shared (all-cores) chunk layout + matmul schedule for one graph.

    Chunks are packed continuously per (supergroup, bank): each core lays
    its slots' bank-b edges end-to-end; the shared chunk count is the max
    over cores, the per-chunk slot list is the union over cores.

    Returns dict:
      sg_list  [(k0, kn)]
      nch      [n_sg][NBANK] shared chunk counts
      c0       [n_sg][NBANK] global first-chunk index
      mms      [n_sg] ordered list of (bank, j, slot_local, start, stop)
      n_chunks total
    """
    counts = plan["counts"]
    ncores, tcore = cfg.n_cores, cfg.t_core
    # Supergroups: full-size bodies, but split the tail into small sgs so
    # the end-of-kernel drain (matmul/copy/W/prelu/store of the last sg)
    # is short and overlaps the final gathers.
    sg_list = []
    k0 = 0
    while k0 < tcore:
        rem = tcore - k0
        if rem > cfg.sg + cfg.sg // 2:
            kn = cfg.sg
        elif rem > cfg.sg:
            kn = (rem + 1) // 2  # two medium tail sgs
        elif rem > 4:
            kn = (rem + 1) // 2  # two small tail sgs
        else:
            kn = rem
        sg_list.append((k0, kn))
        k0 += kn

    nch_all, c0_all, mms_all, spans_all, nidx_all = [], [], [], [], []
    cpos = 0
    for (k0, kn) in sg_list:
        nch_sg = []
        c0_sg = []
        nidx_sg = []
        touches = []  # (slot, bank, j) -> sorted slot-major for PSUM groups
        covered = set()
        for b in range(NBANK):
            cum = np.zeros((ncores, kn + 1), np.int64)
            np.cumsum(counts[:, k0 : k0 + kn, b], axis=1, out=cum[:, 1:])
            maxn = int(max(cum[c, kn] for c in range(ncores)))
            nidx = -(-maxn // 16) * 16  # descriptors charged = num_idxs
            nch = -(-nidx // P)
            nch_sg.append(nch)
            nidx_sg.append(nidx)
            c0_sg.append(cpos)
            cpos += nch
            for j in range(nch):
                lo, hi = j * P, (j + 1) * P
                slots = set()
                for c in range(ncores):
                    for ki in range(kn):
                        if cum[c, ki] < hi and cum[c, ki + 1] > lo:
                            slots.add(ki)
                for ki in sorted(slots):
                    touches.append((ki, b, j))
                    covered.add(ki)
        # slot-major order: each slot's PSUM accumulation group closes
        # before the next one opens in the same PSUM bank
        entries = [(b, j, ki) for (ki, b, j) in sorted(touches)]
        # per-chunk slot span (for the wide one-hot build)
        spans = {}
        for (ki, b, j) in touches:
            lo, hi = spans.get((b, j), (ki, ki))
            spans[(b, j)] = (min(lo, ki), max(hi, ki))
        if sum(nch_sg) == 0:
            # fully empty supergroup: force one pad chunk in bank 0
            nch_sg[0] = 1
            nidx_sg[0] = 16
            for b in range(1, NBANK):
                c0_sg[b] = c0_sg[0] + 1
            cpos += 1
        # zero-coverage slots get one all-zero matmul on the sg's first
        # populated bank's chunk 0 (no core has a matching off there, by
        # construction)
        dummy_bank = next(b for b in range(NBANK) if nch_sg[b] > 0)
        for ki in range(kn):
            if ki not in covered:
                entries.append((dummy_bank, 0, ki))
        # start/stop per slot over entry order
        first, last = {}, {}
        for i, (b, j, ki) in enumerate(entries):
            if ki not in first:
                first[ki] = i
            last[ki] = i
        mms = [(b, j, ki, i == first[ki], i == last[ki])
               for i, (b, j, ki) in enumerate(entries)]
        nch_all.append(nch_sg)
        c0_all.append(c0_sg)
        mms_all.append(mms)
        spans_all.append(spans)
        nidx_all.append(nidx_sg)
    # idx column layout: per (sg, bank) call, nidx/16 int16 columns
    icol_all = []
    icol = 0
    for si in range(len(sg_list)):
        icol_sg = []
        for b in range(NBANK):
            icol_sg.append(icol)
            icol += nidx_all[si][b] // 16
        icol_all.append(icol_sg)
    return dict(sg_list=sg_list, nch=nch_all, c0=c0_all, mms=mms_all,
                spans=spans_all, nidx=nidx_all, icol=icol_all,
                n_chunks=cpos, ncols=icol)


def _fill_core_graph(plan, sched, core, cfg: Config):
    """Build IDX16 (wrapped), OFF and NSE arrays for one core, one graph."""
    n_chunks = sched["n_chunks"]
    idx = np.zeros((n_chunks, P), np.int16)
    off = np.full((n_chunks, P), OFF_PAD, np.float32)
    nse = np.zeros((n_chunks, P), np.float32)
    core_tiles = plan["core_tiles"]
    tile_edges = plan["tile_edges"]
    for si, (k0, kn) in enumerate(sched["sg_list"]):
        for b in range(NBANK):
            nch = sched["nch"][si][b]
            if nch == 0:
                continue
            c0 = sched["c0"][si][b]
            es = np.zeros(nch * P, np.int64)
            eo = np.full(nch * P, OFF_PAD, np.float32)
            en = np.zeros(nch * P, np.float32)
            pos = 0
            for ki in range(kn):
                t = core_tiles[core, k0 + ki]
                if t < 0:
                    continue
                s_rows, s_off, s_nse = tile_edges[t][b]
                nb = len(s_rows)
                es[pos : pos + nb] = s_rows
                eo[pos : pos + nb] = ki * P + s_off
                en[pos : pos + nb] = s_nse
                pos += nb
            idx[c0 : c0 + nch] = es.reshape(nch, P)
            off[c0 : c0 + nch] = eo.reshape(nch, P)
            nse[c0 : c0 + nch] = en.reshape(nch, P)
    # wrap: flat slot i (within a call's first num_idxs slots) ->
    # [i%16, i//16], replicated to 128 partitions. Only num_idxs (<=
    # nch*128, 16-aligned) indices are shipped/charged per call; the
    # remaining tail of the last chunk is never gathered and its off
    # stays OFF_PAD.
    idx_w = np.zeros((P, sched["ncols"]), np.int16)
    for si in range(len(sched["sg_list"])):
        for b in range(NBANK):
            nidx = sched["nidx"][si][b]
            if nidx == 0:
                continue
            c0 = sched["c0"][si][b]
            icol = sched["icol"][si][b]
            flat = idx[c0 : c0 + sched["nch"][si][b]].reshape(-1)[:nidx]
            blk = flat.reshape(-1, 16).T  # [16, nidx/16]
            idx_w[:, icol : icol + nidx // 16] = np.tile(blk, (8, 1))
    # interleave off/nse per supergroup block ([off_cols | nse_cols]) so
    # the per-sg DMA load is one >=512B-per-partition transfer (no 2x
    # small-descriptor penalty)
    offT, nseT = off.T, nse.T
    onse = np.empty((P, 2 * n_chunks), np.float32)
    for si in range(len(sched["sg_list"])):
        nchs = sched["nch"][si]
        c0_sg = min(sched["c0"][si][b] for b in range(NBANK) if nchs[b] > 0)
        sgc = sum(nchs)
        onse[:, 2 * c0_sg : 2 * c0_sg + sgc] = offT[:, c0_sg : c0_sg + sgc]
        onse[:, 2 * c0_sg + sgc : 2 * (c0_sg + sgc)] = (
            nseT[:, c0_sg : c0_sg + sgc])
    return idx_w, onse  # [P, 2*n_chunks]


def preprocess(feats, W, b, prelu_a, src_pos, dst_pos, src_neg, dst_neg,
               cfg: Config):
    n, ncores = cfg.n_nodes, cfg.n_cores
    feats = np.asarray(feats, np.float32)
    W = np.asarray(W, np.float32)
    b = np.asarray(b, np.float32)
    prelu_a = np.asarray(prelu_a, np.float32)

    featsr = np.zeros((cfg.n_pad, D), BF16)  # row-major, padded, bf16
    featsr[:n] = feats.astype(BF16)

    plans, scheds = [], []
    for src, dst in ((src_pos, dst_pos), (src_neg, dst_neg)):
        src = np.asarray(src, np.int64)
        dst = np.asarray(dst, np.int64)
        deg_out = np.bincount(src, minlength=n).astype(np.float32)
        deg_in = np.bincount(dst, minlength=n).astype(np.float32)
        ns = np.where(deg_out > 0, 1.0 / np.sqrt(np.maximum(deg_out, 1.0)),
                      0.0).astype(np.float32)
        nd = np.where(deg_in > 0, 1.0 / np.sqrt(np.maximum(deg_in, 1.0)),
                      0.0).astype(np.float32)
        nse_edge = ns[src] * nd[dst]
        plan = _plan_graph(src, dst, nse_edge, cfg)
        plans.append(plan)
        scheds.append(_schedule_graph(plan, cfg))

    ramp = np.tile(np.arange(cfg.sg * P, dtype=np.int16), (P, 1))
    a_rep = np.full((P, 1), float(prelu_a.reshape(-1)[0]), np.float32)
    b_rep = np.tile(b.reshape(1, D), (P, 4)).astype(np.float32)

    in_maps = []
    for core in range(ncores):
        iw_p, onse_p = _fill_core_graph(plans[0], scheds[0], core, cfg)
        iw_n, onse_n = _fill_core_graph(plans[1], scheds[1], core, cfg)
        in_maps.append({
            "featsr": featsr,
            "w_in": W,
            "a_rep": a_rep,
            "b_rep": b_rep,
            "idx_in": np.concatenate([iw_p, iw_n], axis=1),
            "onse_in": np.concatenate([onse_p, onse_n], axis=1),
            "ramp_in": ramp,
        })
    meta = {
        "scheds": scheds,
        "use_bias": bool(np.any(b != 0.0)),
    }
    return in_maps, plans, meta


# --------------------------------------------------------------------------
# Device kernel builder
# --------------------------------------------------------------------------
def build_kernel(nc, tc, cfg: Config, meta):
    from contextlib import ExitStack

    import concourse.mybir as mybir

    f32 = mybir.dt.float32
    bf16 = mybir.dt.bfloat16
    i16 = mybir.dt.int16
    Alu = mybir.AluOpType
    Act = mybir.ActivationFunctionType

    npad = cfg.n_pad
    scheds = meta["scheds"]
    use_bias = meta["use_bias"]
    n_chunks = [scheds[g]["n_chunks"] for g in range(2)]
    ncols = [scheds[g]["ncols"] for g in range(2)]
    n_sg = len(scheds[0]["sg_list"])
    assert len(scheds[1]["sg_list"]) == n_sg

    featsr = nc.dram_tensor("featsr", [npad, D], bf16,
                            kind="ExternalInput").ap()
    w_in = nc.dram_tensor("w_in", [P, D], f32, kind="ExternalInput").ap()
    a_rep = nc.dram_tensor("a_rep", [P, 1], f32, kind="ExternalInput").ap()
    b_rep = nc.dram_tensor("b_rep", [P, 4 * D], f32, kind="ExternalInput").ap()
    idx_in = nc.dram_tensor("idx_in", [P, sum(ncols)], i16,
                            kind="ExternalInput").ap()
    onse_in = nc.dram_tensor("onse_in", [P, 2 * sum(n_chunks)], f32,
                             kind="ExternalInput").ap()
    ramp_in = nc.dram_tensor("ramp_in", [P, cfg.sg * P], i16,
                             kind="ExternalInput").ap()
    out = nc.dram_tensor("out", [2, n_sg, P, cfg.sg * D], bf16,
                         kind="ExternalOutput").ap()

    with ExitStack() as ctx:
        const = ctx.enter_context(tc.tile_pool(name="const", bufs=1))
        gpool = ctx.enter_context(tc.tile_pool(name="gpool", bufs=cfg.gbufs))
        ipool = ctx.enter_context(tc.tile_pool(name="ipool", bufs=cfg.ipbufs))
        ohpool = ctx.enter_context(tc.tile_pool(name="ohpool", bufs=24))
        aggpool = ctx.enter_context(tc.tile_pool(name="aggpool", bufs=4))
        tpool = ctx.enter_context(tc.tile_pool(name="tpool", bufs=4))
        spool = ctx.enter_context(tc.tile_pool(name="spool", bufs=3))
        ppool = ctx.enter_context(tc.tile_pool(name="ppool", bufs=cfg.ppbufs,
                                               space="PSUM"))
        hpool = ctx.enter_context(tc.tile_pool(name="hpool", bufs=cfg.hpbufs,
                                               space="PSUM"))

        # ---- constants ----
        w_sb = const.tile([P, D], bf16)
        nc.gpsimd.dma_start(out=w_sb[:], in_=w_in)  # f32 -> bf16 cast DMA
        ramp_sb = const.tile([P, cfg.sg * P], i16)
        nc.sync.dma_start(out=ramp_sb[:], in_=ramp_in)
        a_sb = const.tile([P, 1], f32)
        nc.sync.dma_start(out=a_sb[:], in_=a_rep)
        if use_bias:
            b_sb = const.tile([P, 4 * D], f32)
            nc.sync.dma_start(out=b_sb[:], in_=b_rep)

        max_sgc = max(sum(scheds[g]["nch"][si]) for g in range(2)
                      for si in range(n_sg))

        # ---- gather + weighted one-hot segment-sum + @W + prelu ----
        col_base = [0, ncols[0]]          # idx column offset per graph
        chk_base = [0, 2 * n_chunks[0]]   # onse column offset per graph
        # interleave the two graphs' supergroups so one graph's gathers fill
        # DMA while the other's PSUM chain drains
        jobs = []
        for si in range(n_sg):
            for g in range(2):
                jobs.append((g, si))
        ecnt = 0
        for (g, si) in jobs:
            sch = scheds[g]
            (k0, kn) = sch["sg_list"][si]
            nchs = sch["nch"][si]
            c0s = sch["c0"][si]
            c0_sg = min(c0s[b] for b in range(NBANK) if nchs[b] > 0)
            sg_chunks = sum(nchs)
            nidxs = sch["nidx"][si]
            icols = sch["icol"][si]
            icol_sg = icols[0]
            icol_w = sum(nidxs) // 16
            gt = gpool.tile([P, max_sgc, D], bf16, tag="gather")
            it = ipool.tile([P, icol_w], i16, tag="gidx")
            nc.sync.dma_start(
                out=it[:],
                in_=idx_in[:, col_base[g] + icol_sg :
                           col_base[g] + icol_sg + icol_w])
            oet = ipool.tile([P, 2 * sg_chunks], f32, tag="gonse")
            nc.sync.dma_start(
                out=oet[:],
                in_=onse_in[:, chk_base[g] + 2 * c0_sg :
                            chk_base[g] + 2 * (c0_sg + sg_chunks)])
            ot = oet[:, :sg_chunks]
            et = oet[:, sg_chunks:]
            for b in range(NBANK):
                nch = nchs[b]
                if nch == 0:
                    continue
                lo = c0s[b] - c0_sg
                ilo = icols[b] - icol_sg
                nidx = nidxs[b]
                bank_rows = min(cfg.bank_rows, npad - b * cfg.bank_rows)
                nc.gpsimd.dma_gather(
                    out_ap=gt[:, lo : lo + nch, :],
                    in_ap=featsr[b * cfg.bank_rows :
                                 b * cfg.bank_rows + bank_rows, :],
                    idxs_ap=it[:, ilo : ilo + nidx // 16],
                    num_idxs=nidx, num_idxs_reg=nidx,
                    elem_size=D, single_packet=False)
            nquad = (kn + 3) // 4
            psTs = [ppool.tile([P, 4 * D], f32, tag="psT", name="psT")
                    for _ in range(nquad)]
            spans = sch["spans"][si]
            oh_cache = {}
            for (b, j, ki, st, sp) in sch["mms"][si]:
                lo = c0s[b] - c0_sg + j
                pv = min(P, nidxs[b] - j * P)  # valid rows in this chunk
                klo, khi = spans.get((b, j), (ki, ki))
                if ki < klo or ki > khi:
                    # dummy zero matmul for an uncovered slot: one-off
                    # narrow one-hot (off values there never match ki)
                    oh = ohpool.tile([P, D], bf16, tag="ohw", name="ohw")
                    nc.vector.tensor_scalar(
                        out=oh[:], in0=ramp_sb[:, ki * P : (ki + 1) * P],
                        scalar1=ot[:, lo : lo + 1],
                        scalar2=et[:, lo : lo + 1],
                        op0=Alu.is_equal, op1=Alu.mult)
                    rhs = oh[:pv, :]
                else:
                    if (b, j) not in oh_cache:
                        span = khi - klo + 1
                        ohw = ohpool.tile([P, span * D], bf16, tag="ohw",
                                          name="ohw")
                        eng = nc.vector
                        if cfg.oh_gpsimd_mod and (
                                ecnt % cfg.oh_gpsimd_mod == 0):
                            eng = nc.gpsimd
                        ecnt += 1
                        eng.tensor_scalar(
                            out=ohw[:],
                            in0=ramp_sb[:, klo * P : (khi + 1) * P],
                            scalar1=ot[:, lo : lo + 1],
                            scalar2=et[:, lo : lo + 1],
                            op0=Alu.is_equal, op1=Alu.mult)
                        oh_cache[(b, j)] = ohw
                    ohw = oh_cache[(b, j)]
                    rhs = ohw[:pv, (ki - klo) * D : (ki - klo + 1) * D]
                q, r = divmod(ki, 4)
                nc.tensor.matmul(
                    out=psTs[q][:, r * D : (r + 1) * D],
                    lhsT=gt[:pv, lo, :], rhs=rhs, start=st, stop=sp)
            stg = spool.tile([P, kn * D], bf16, tag="stg")
            for q in range(nquad):
                kq = min(4, kn - 4 * q)
                aggsb = aggpool.tile([P, 4 * D], bf16, tag="aggsb")
                nc.scalar.activation(out=aggsb[:, : kq * D],
                                     in_=psTs[q][:, : kq * D],
                                     func=Act.Copy)
                hps = hpool.tile([P, 4 * D], f32)
                for r in range(kq):
                    nc.tensor.matmul(out=hps[:, r * D : (r + 1) * D],
                                     lhsT=aggsb[:, r * D : (r + 1) * D],
                                     rhs=w_sb[:], start=True, stop=True)
                ss = stg[:, 4 * q * D : (4 * q + kq) * D]
                if use_bias:
                    hb2 = tpool.tile([P, 4 * D], f32, tag="hb2")
                    nc.vector.tensor_tensor(out=hb2[:, : kq * D],
                                            in0=hps[:, : kq * D],
                                            in1=b_sb[:, : kq * D], op=Alu.add)
                    neg = tpool.tile([P, 4 * D], f32, tag="neg")
                    nc.vector.tensor_scalar(
                        out=neg[:, : kq * D], in0=hb2[:, : kq * D],
                        scalar1=0.0, scalar2=a_sb[:, :1],
                        op0=Alu.min, op1=Alu.mult)
                    pos = tpool.tile([P, 4 * D], f32, tag="pos")
                    nc.vector.tensor_scalar(
                        out=pos[:, : kq * D], in0=hb2[:, : kq * D],
                        scalar1=0.0, scalar2=None, op0=Alu.max)
                    nc.vector.tensor_tensor(out=ss, in0=neg[:, : kq * D],
                                            in1=pos[:, : kq * D], op=Alu.add)
                elif cfg.act_prelu:
                    nc.scalar.activation(
                        out=ss, in_=hps[:, : kq * D], func=Act.Prelu,
                        alpha=a_sb[:, :1])
                else:
                    neg = tpool.tile([P, 4 * D], f32, tag="neg")
                    nc.vector.tensor_scalar(
                        out=neg[:, : kq * D], in0=hps[:, : kq * D],
                        scalar1=0.0, scalar2=a_sb[:, :1],
                        op0=Alu.min, op1=Alu.mult)
                    pos = tpool.tile([P, 4 * D], f32, tag="pos")
                    nc.vector.tensor_scalar(
                        out=pos[:, : kq * D], in0=hps[:, : kq * D],
                        scalar1=0.0, scalar2=None, op0=Alu.max)
                    nc.vector.tensor_tensor(out=ss, in0=neg[:, : kq * D],
                                            in1=pos[:, : kq * D], op=Alu.add)
            nc.sync.dma_start(out=out[g, si, :, : kn * D], in_=stg[:])
    return out


# --------------------------------------------------------------------------
# Driver
# --------------------------------------------------------------------------
def _build_program(cfg: Config, meta):
    import concourse.bacc as bacc
    import concourse.tile as tile

    nc = bacc.Bacc("TRN2", target_bir_lowering=False, debug=False,
                   enable_asserts=False, num_devices=cfg.n_cores)
    with tile.TileContext(nc) as tc:
        build_kernel(nc, tc, cfg, meta)
    nc.compile()
    return nc


def _unscramble(results, plans, meta, cfg: Config):
    n = cfg.n_nodes
    full = np.zeros((2, n, D), np.float32)
    for g in range(2):
        sched = meta["scheds"][g]
        ct_all = plans[g]["core_tiles"]
        for core in range(cfg.n_cores):
            oc = np.asarray(results[core]["out"], dtype=np.float32)
            for si, (k0, kn) in enumerate(sched["sg_list"]):
                for ki in range(kn):
                    t = int(ct_all[core, k0 + ki])
                    if t < 0:
                        continue
                    r0 = t * P
                    r1 = min(r0 + P, n)
                    full[g, r0:r1] = oc[g, si, : r1 - r0,
                                        ki * D : (ki + 1) * D]
    return full


_PROGRAM_CACHE = {}


def _sched_key(sched):
    return (tuple(tuple(x) for x in sched["nch"]),
            tuple(mm for sgm in sched["mms"] for mm in sgm))


def run(inputs, cfg: Config, trace=False):
    from concourse.bass_utils import run_bass_kernel_spmd

    in_maps, plans, meta = preprocess(
        inputs["feats"], inputs["W"], inputs["b"], inputs["prelu_a"],
        inputs["src_pos"], inputs["dst_pos"],
        inputs["src_neg"], inputs["dst_neg"], cfg)

    key = (cfg.n_nodes, cfg.n_cores, cfg.sg,
           cfg.act_prelu, cfg.oh_gpsimd_mod, cfg.gbufs,
           _sched_key(meta["scheds"][0]), _sched_key(meta["scheds"][1]),
           meta["use_bias"])
    nc = _PROGRAM_CACHE.get(key)
    if nc is None:
        nc = _build_program(cfg, meta)
        _PROGRAM_CACHE[key] = nc

    kwargs = {}
    if trace:
        kwargs = dict(trace=True, tmpdir=tempfile.mkdtemp(prefix="bgc_trace_"))
    res = run_bass_kernel_spmd(nc, in_maps, core_ids=list(range(cfg.n_cores)),
                               **kwargs)
    full = _unscramble(res.results, plans, meta, cfg)
    return full, res


def kernel(**inputs) -> np.ndarray:
    cfg = Config()
    full, _ = run(inputs, cfg)
    return full


# revision 36
# speedup vs baseline: 1.0185x; 1.0077x over previous
"""Bass/Trainium2 kernel for BiGraphContrastLayer (GNN message passing).

Computes, for two edge lists (pos/neg) over the same node features:
    h_g = PReLU( D_in^-1/2 A_g D_out^-1/2 feats @ W + b )
returning stack([h_pos, h_neg]) of shape [2, N, Dout].

Strategy (8 NeuronCores, SPMD, no collectives), using the linearity
   (D_in^-1/2 A D_out^-1/2 feats) @ W = (D_in^-1/2 A D_out^-1/2 feats) W:

  No y-phase: dma_gather (int16 idx, 4 src-row banks of <=25088 rows)
  pulls RAW bf16 feats rows straight from the (host-cast, padded) input.
  Edges are bucketed by dst tile (slot), dealt to cores, and packed
  CONTINUOUSLY per (supergroup, bank) into 128-row chunks — a chunk may
  span several dst slots; per (chunk, slot) a weighted one-hot
  (rhs[p, j] = ns[src_p]*nd[dst_p] iff slot-relative dst position match,
  built on DVE via is_equal(int16 ramp slice, f32 off) + mult) matmul
  accumulates into the slot's quarter of a quad [128, 512] PSUM bank
  aggT[feat, dst]. Per quad: one cast copy aggT -> SBUF bf16, per slot a
  matmul (lhsT=aggT slice, rhs=W) -> h PSUM quad, one PReLU on ScalarE,
  bf16 store in a p-major layout (contiguous per partition).

  All 8 cores share one instruction stream: the chunk/matmul schedule is
  the UNION over cores (max chunk counts, union slot spans); cores
  lacking edges for a scheduled (chunk, slot) build an all-zero one-hot.

  Host does index/metadata work only: degree bincounts -> per-edge norm
  weights, sorting, bucketing, dealing, packing, int16 wrapped gather
  indices, replicating the small W/b/prelu params per the sharding hint.
"""

import math
import tempfile
from dataclasses import dataclass

import numpy as np

P = 128   # partitions
D = 128   # feature dim (Din == Dout == 128)
NBANK = 4
BF16 = np.dtype("bfloat16")
OFF_PAD = 4096.0  # off value matching no ramp slice


# --------------------------------------------------------------------------
# Config
# --------------------------------------------------------------------------
@dataclass
class Config:
    n_nodes: int = 100000
    n_cores: int = 8
    sg: int = 16       # dst-tile slots per supergroup (multiple of 4 best)
    oh_gpsimd_mod: int = 0    # every Nth one-hot build goes to GpSimd (0=off)
    act_prelu: bool = True    # final PReLU on ScalarE (not in sim)
    gbufs: int = 3            # gather buffer count
    ipbufs: int = 4           # idx buffer count
    ppbufs: int = 6           # PSUM quad accumulator banks
    hpbufs: int = 2           # PSUM h banks

    @property
    def t_global(self) -> int:
        return math.ceil(self.n_nodes / P)

    @property
    def n_pad(self) -> int:
        return self.t_global * P

    @property
    def t_core(self) -> int:
        return math.ceil(self.t_global / self.n_cores)

    @property
    def bank_tiles(self) -> int:
        return math.ceil(self.t_global / NBANK)

    @property
    def bank_rows(self) -> int:
        return self.bank_tiles * P

    @property
    def n_sg(self) -> int:
        return math.ceil(self.t_core / self.sg)


# --------------------------------------------------------------------------
# Host-side preprocessing (integer index / edge-weight metadata only)
# --------------------------------------------------------------------------
def _plan_graph(src, dst, nse_edge, cfg: Config):
    """Bucket edges by dst tile, sort by src within tile, bank-split, and
    deal tiles to cores (snake by total edge count for balance).

    Returns dict with:
      core_tiles  [n_cores, t_core]  global tile id per slot (-1 null)
      counts      [n_cores, t_core, NBANK] per-slot-bank edge counts
      tile_edges  list per global tile, per bank: (src_local, dstoff, nse)
    """
    tg, ncores, tcore = cfg.t_global, cfg.n_cores, cfg.t_core
    order = np.argsort(dst, kind="stable")
    src_s = src[order]
    dst_s = dst[order]
    nse_s = nse_edge[order]
    tile_cnt = np.bincount(dst_s // P, minlength=tg)
    starts = np.zeros(tg + 1, np.int64)
    np.cumsum(tile_cnt, out=starts[1:])

    bank_of = src_s // cfg.bank_rows
    tile_edges = []
    for t in range(tg):
        e0, e1 = int(starts[t]), int(starts[t + 1])
        per_bank = []
        for b in range(NBANK):
            m = bank_of[e0:e1] == b
            per_bank.append((
                (src_s[e0:e1][m] - b * cfg.bank_rows).astype(np.int64),
                (dst_s[e0:e1][m] % P).astype(np.int64),
                nse_s[e0:e1][m].astype(np.float32),
            ))
        tile_edges.append(per_bank)

    # Deal tiles by descending total edges; within each rank-group of
    # ncores tiles, greedily give each tile to the core whose running
    # per-bank supergroup sums stay smallest (bank-aware LPT) — the
    # shared per-(sg, bank) chunk count is the max over cores, so
    # minimizing the max per-bank deficit minimizes gather padding.
    bank_cnt = np.zeros((tg, NBANK), np.int64)
    for t in range(tg):
        for b in range(NBANK):
            bank_cnt[t, b] = len(tile_edges[t][b][0])
    keys = np.argsort(tile_cnt, kind="stable")[::-1]
    core_tiles = np.full((ncores, tcore), -1, np.int64)
    run = np.zeros((ncores, NBANK), np.int64)
    for k in range(tcore):
        if k % cfg.sg == 0:
            run[:] = 0  # new supergroup
        grp = keys[k * ncores : (k + 1) * ncores]
        free = list(range(ncores))
        for t in grp:
            proj = run[free] + bank_cnt[t]
            i = int(np.argmin(proj.max(axis=1) + 1e-3 * proj.sum(axis=1)))
            c = free.pop(i)
            core_tiles[c, k] = t
            run[c] += bank_cnt[t]

    # Refinement: within each supergroup, greedily swap same-slot tile
    # assignments between cores while it reduces sum_b max_c of the
    # per-bank supergroup sums (the quantity the shared gather pays for).
    def _cnt(t):
        return bank_cnt[t] if t >= 0 else np.zeros(NBANK, np.int64)

    for k0 in range(0, tcore, cfg.sg):
        kn = min(cfg.sg, tcore - k0)
        S = np.zeros((ncores, NBANK), np.int64)
        for c in range(ncores):
            for k in range(k0, k0 + kn):
                S[c] += _cnt(core_tiles[c, k])
        for _ in range(4):
            improved = False
            base = S.max(axis=0).sum()
            for k in range(k0, k0 + kn):
                for c1 in range(ncores):
                    for c2 in range(c1 + 1, ncores):
                        d1 = _cnt(core_tiles[c2, k]) - _cnt(core_tiles[c1, k])
                        S[c1] += d1
                        S[c2] -= d1
                        new = S.max(axis=0).sum()
                        if new < base:
                            base = new
                            core_tiles[c1, k], core_tiles[c2, k] = (
                                core_tiles[c2, k], core_tiles[c1, k])
                            improved = True
                        else:
                            S[c1] -= d1
                            S[c2] += d1
            if not improved:
                break

    counts = np.zeros((ncores, tcore, NBANK), np.int64)
    for c in range(ncores):
        for k in range(tcore):
            t = core_tiles[c, k]
            if t < 0:
                continue
            for b in range(NBANK):
                counts[c, k, b] = len(tile_edges[t][b][0])
    return dict(core_tiles=core_tiles, counts=counts, tile_edges=tile_edges)


def _schedule_graph(plan, cfg: Config):
    """Shared (all-cores) chunk layout + matmul schedule for one graph.

    Chunks are packed continuously per (supergroup, bank): each core lays
    its slots' bank-b edges end-to-end; the shared chunk count is the max
    over cores, the per-chunk slot list is the union over cores.

    Returns dict:
      sg_list  [(k0, kn)]
      nch      [n_sg][NBANK] shared chunk counts
      c0       [n_sg][NBANK] global first-chunk index
      mms      [n_sg] ordered list of (bank, j, slot_local, start, stop)
      n_chunks total
    """
    counts = plan["counts"]
    ncores, tcore = cfg.n_cores, cfg.t_core
    # Supergroups: full-size bodies, but split the tail into small sgs so
    # the end-of-kernel drain (matmul/copy/W/prelu/store of the last sg)
    # is short and overlaps the final gathers.
    sg_list = []
    k0 = 0
    while k0 < tcore:
        rem = tcore - k0
        if rem > cfg.sg + cfg.sg // 2:
            kn = cfg.sg
        elif rem > cfg.sg:
            kn = (rem + 2) // 3  # three medium tail sgs
        elif rem > 4:
            kn = (rem + 1) // 2  # two small tail sgs
        else:
            kn = rem
        sg_list.append((k0, kn))
        k0 += kn

    nch_all, c0_all, mms_all, spans_all, nidx_all = [], [], [], [], []
    cpos = 0
    for (k0, kn) in sg_list:
        nch_sg = []
        c0_sg = []
        nidx_sg = []
        touches = []  # (slot, bank, j) -> sorted slot-major for PSUM groups
        covered = set()
        for b in range(NBANK):
            cum = np.zeros((ncores, kn + 1), np.int64)
            np.cumsum(counts[:, k0 : k0 + kn, b], axis=1, out=cum[:, 1:])
            maxn = int(max(cum[c, kn] for c in range(ncores)))
            nidx = -(-maxn // 16) * 16  # descriptors charged = num_idxs
            nch = -(-nidx // P)
            nch_sg.append(nch)
            nidx_sg.append(nidx)
            c0_sg.append(cpos)
            cpos += nch
            for j in range(nch):
                lo, hi = j * P, (j + 1) * P
                slots = set()
                for c in range(ncores):
                    for ki in range(kn):
                        if cum[c, ki] < hi and cum[c, ki + 1] > lo:
                            slots.add(ki)
                for ki in sorted(slots):
                    touches.append((ki, b, j))
                    covered.add(ki)
        # slot-major order: each slot's PSUM accumulation group closes
        # before the next one opens in the same PSUM bank
        entries = [(b, j, ki) for (ki, b, j) in sorted(touches)]
        # per-chunk slot span (for the wide one-hot build)
        spans = {}
        for (ki, b, j) in touches:
            lo, hi = spans.get((b, j), (ki, ki))
            spans[(b, j)] = (min(lo, ki), max(hi, ki))
        if sum(nch_sg) == 0:
            # fully empty supergroup: force one pad chunk in bank 0
            nch_sg[0] = 1
            nidx_sg[0] = 16
            for b in range(1, NBANK):
                c0_sg[b] = c0_sg[0] + 1
            cpos += 1
        # zero-coverage slots get one all-zero matmul on the sg's first
        # populated bank's chunk 0 (no core has a matching off there, by
        # construction)
        dummy_bank = next(b for b in range(NBANK) if nch_sg[b] > 0)
        for ki in range(kn):
            if ki not in covered:
                entries.append((dummy_bank, 0, ki))
        # start/stop per slot over entry order
        first, last = {}, {}
        for i, (b, j, ki) in enumerate(entries):
            if ki not in first:
                first[ki] = i
            last[ki] = i
        mms = [(b, j, ki, i == first[ki], i == last[ki])
               for i, (b, j, ki) in enumerate(entries)]
        nch_all.append(nch_sg)
        c0_all.append(c0_sg)
        mms_all.append(mms)
        spans_all.append(spans)
        nidx_all.append(nidx_sg)
    # idx column layout: per (sg, bank) call, nidx/16 int16 columns
    icol_all = []
    icol = 0
    for si in range(len(sg_list)):
        icol_sg = []
        for b in range(NBANK):
            icol_sg.append(icol)
            icol += nidx_all[si][b] // 16
        icol_all.append(icol_sg)
    return dict(sg_list=sg_list, nch=nch_all, c0=c0_all, mms=mms_all,
                spans=spans_all, nidx=nidx_all, icol=icol_all,
                n_chunks=cpos, ncols=icol)


def _fill_core_graph(plan, sched, core, cfg: Config):
    """Build IDX16 (wrapped), OFF and NSE arrays for one core, one graph."""
    n_chunks = sched["n_chunks"]
    idx = np.zeros((n_chunks, P), np.int16)
    off = np.full((n_chunks, P), OFF_PAD, np.float32)
    nse = np.zeros((n_chunks, P), np.float32)
    core_tiles = plan["core_tiles"]
    tile_edges = plan["tile_edges"]
    for si, (k0, kn) in enumerate(sched["sg_list"]):
        for b in range(NBANK):
            nch = sched["nch"][si][b]
            if nch == 0:
                continue
            c0 = sched["c0"][si][b]
            es = np.zeros(nch * P, np.int64)
            eo = np.full(nch * P, OFF_PAD, np.float32)
            en = np.zeros(nch * P, np.float32)
            pos = 0
            for ki in range(kn):
                t = core_tiles[core, k0 + ki]
                if t < 0:
                    continue
                s_rows, s_off, s_nse = tile_edges[t][b]
                nb = len(s_rows)
                es[pos : pos + nb] = s_rows
                eo[pos : pos + nb] = ki * P + s_off
                en[pos : pos + nb] = s_nse
                pos += nb
            idx[c0 : c0 + nch] = es.reshape(nch, P)
            off[c0 : c0 + nch] = eo.reshape(nch, P)
            nse[c0 : c0 + nch] = en.reshape(nch, P)
    # wrap: flat slot i (within a call's first num_idxs slots) ->
    # [i%16, i//16], replicated to 128 partitions. Only num_idxs (<=
    # nch*128, 16-aligned) indices are shipped/charged per call; the
    # remaining tail of the last chunk is never gathered and its off
    # stays OFF_PAD.
    idx_w = np.zeros((P, sched["ncols"]), np.int16)
    for si in range(len(sched["sg_list"])):
        for b in range(NBANK):
            nidx = sched["nidx"][si][b]
            if nidx == 0:
                continue
            c0 = sched["c0"][si][b]
            icol = sched["icol"][si][b]
            flat = idx[c0 : c0 + sched["nch"][si][b]].reshape(-1)[:nidx]
            blk = flat.reshape(-1, 16).T  # [16, nidx/16]
            idx_w[:, icol : icol + nidx // 16] = np.tile(blk, (8, 1))
    # interleave off/nse per supergroup block ([off_cols | nse_cols]) so
    # the per-sg DMA load is one >=512B-per-partition transfer (no 2x
    # small-descriptor penalty)
    offT, nseT = off.T, nse.T
    onse = np.empty((P, 2 * n_chunks), np.float32)
    for si in range(len(sched["sg_list"])):
        nchs = sched["nch"][si]
        c0_sg = min(sched["c0"][si][b] for b in range(NBANK) if nchs[b] > 0)
        sgc = sum(nchs)
        onse[:, 2 * c0_sg : 2 * c0_sg + sgc] = offT[:, c0_sg : c0_sg + sgc]
        onse[:, 2 * c0_sg + sgc : 2 * (c0_sg + sgc)] = (
            nseT[:, c0_sg : c0_sg + sgc])
    return idx_w, onse  # [P, 2*n_chunks]


def preprocess(feats, W, b, prelu_a, src_pos, dst_pos, src_neg, dst_neg,
               cfg: Config):
    n, ncores = cfg.n_nodes, cfg.n_cores
    feats = np.asarray(feats, np.float32)
    W = np.asarray(W, np.float32)
    b = np.asarray(b, np.float32)
    prelu_a = np.asarray(prelu_a, np.float32)

    featsr = np.zeros((cfg.n_pad, D), BF16)  # row-major, padded, bf16
    featsr[:n] = feats.astype(BF16)

    plans, scheds = [], []
    for src, dst in ((src_pos, dst_pos), (src_neg, dst_neg)):
        src = np.asarray(src, np.int64)
        dst = np.asarray(dst, np.int64)
        deg_out = np.bincount(src, minlength=n).astype(np.float32)
        deg_in = np.bincount(dst, minlength=n).astype(np.float32)
        ns = np.where(deg_out > 0, 1.0 / np.sqrt(np.maximum(deg_out, 1.0)),
                      0.0).astype(np.float32)
        nd = np.where(deg_in > 0, 1.0 / np.sqrt(np.maximum(deg_in, 1.0)),
                      0.0).astype(np.float32)
        nse_edge = ns[src] * nd[dst]
        plan = _plan_graph(src, dst, nse_edge, cfg)
        plans.append(plan)
        scheds.append(_schedule_graph(plan, cfg))

    ramp = np.tile(np.arange(cfg.sg * P, dtype=np.int16), (P, 1))
    a_rep = np.full((P, 1), float(prelu_a.reshape(-1)[0]), np.float32)
    b_rep = np.tile(b.reshape(1, D), (P, 4)).astype(np.float32)

    in_maps = []
    for core in range(ncores):
        iw_p, onse_p = _fill_core_graph(plans[0], scheds[0], core, cfg)
        iw_n, onse_n = _fill_core_graph(plans[1], scheds[1], core, cfg)
        in_maps.append({
            "featsr": featsr,
            "w_in": W,
            "a_rep": a_rep,
            "b_rep": b_rep,
            "idx_in": np.concatenate([iw_p, iw_n], axis=1),
            "onse_in": np.concatenate([onse_p, onse_n], axis=1),
            "ramp_in": ramp,
        })
    meta = {
        "scheds": scheds,
        "use_bias": bool(np.any(b != 0.0)),
    }
    return in_maps, plans, meta


# --------------------------------------------------------------------------
# Device kernel builder
# --------------------------------------------------------------------------
def build_kernel(nc, tc, cfg: Config, meta):
    from contextlib import ExitStack

    import concourse.mybir as mybir

    f32 = mybir.dt.float32
    bf16 = mybir.dt.bfloat16
    i16 = mybir.dt.int16
    Alu = mybir.AluOpType
    Act = mybir.ActivationFunctionType

    npad = cfg.n_pad
    scheds = meta["scheds"]
    use_bias = meta["use_bias"]
    n_chunks = [scheds[g]["n_chunks"] for g in range(2)]
    ncols = [scheds[g]["ncols"] for g in range(2)]
    n_sg = len(scheds[0]["sg_list"])
    assert len(scheds[1]["sg_list"]) == n_sg

    featsr = nc.dram_tensor("featsr", [npad, D], bf16,
                            kind="ExternalInput").ap()
    w_in = nc.dram_tensor("w_in", [P, D], f32, kind="ExternalInput").ap()
    a_rep = nc.dram_tensor("a_rep", [P, 1], f32, kind="ExternalInput").ap()
    b_rep = nc.dram_tensor("b_rep", [P, 4 * D], f32, kind="ExternalInput").ap()
    idx_in = nc.dram_tensor("idx_in", [P, sum(ncols)], i16,
                            kind="ExternalInput").ap()
    onse_in = nc.dram_tensor("onse_in", [P, 2 * sum(n_chunks)], f32,
                             kind="ExternalInput").ap()
    ramp_in = nc.dram_tensor("ramp_in", [P, cfg.sg * P], i16,
                             kind="ExternalInput").ap()
    out = nc.dram_tensor("out", [2, n_sg, P, cfg.sg * D], bf16,
                         kind="ExternalOutput").ap()

    with ExitStack() as ctx:
        const = ctx.enter_context(tc.tile_pool(name="const", bufs=1))
        gpool = ctx.enter_context(tc.tile_pool(name="gpool", bufs=cfg.gbufs))
        ipool = ctx.enter_context(tc.tile_pool(name="ipool", bufs=cfg.ipbufs))
        ohpool = ctx.enter_context(tc.tile_pool(name="ohpool", bufs=24))
        aggpool = ctx.enter_context(tc.tile_pool(name="aggpool", bufs=4))
        tpool = ctx.enter_context(tc.tile_pool(name="tpool", bufs=4))
        spool = ctx.enter_context(tc.tile_pool(name="spool", bufs=3))
        ppool = ctx.enter_context(tc.tile_pool(name="ppool", bufs=cfg.ppbufs,
                                               space="PSUM"))
        hpool = ctx.enter_context(tc.tile_pool(name="hpool", bufs=cfg.hpbufs,
                                               space="PSUM"))

        # ---- constants ----
        w_sb = const.tile([P, D], bf16)
        nc.gpsimd.dma_start(out=w_sb[:], in_=w_in)  # f32 -> bf16 cast DMA
        ramp_sb = const.tile([P, cfg.sg * P], i16)
        nc.gpsimd.dma_start(out=ramp_sb[:], in_=ramp_in)
        a_sb = const.tile([P, 1], f32)
        nc.gpsimd.dma_start(out=a_sb[:], in_=a_rep)
        if use_bias:
            b_sb = const.tile([P, 4 * D], f32)
            nc.sync.dma_start(out=b_sb[:], in_=b_rep)

        max_sgc = max(sum(scheds[g]["nch"][si]) for g in range(2)
                      for si in range(n_sg))

        # ---- gather + weighted one-hot segment-sum + @W + prelu ----
        col_base = [0, ncols[0]]          # idx column offset per graph
        chk_base = [0, 2 * n_chunks[0]]   # onse column offset per graph
        # interleave the two graphs' supergroups so one graph's gathers fill
        # DMA while the other's PSUM chain drains
        jobs = []
        for si in range(n_sg):
            for g in range(2):
                jobs.append((g, si))
        ecnt = 0
        for (g, si) in jobs:
            sch = scheds[g]
            (k0, kn) = sch["sg_list"][si]
            nchs = sch["nch"][si]
            c0s = sch["c0"][si]
            c0_sg = min(c0s[b] for b in range(NBANK) if nchs[b] > 0)
            sg_chunks = sum(nchs)
            nidxs = sch["nidx"][si]
            icols = sch["icol"][si]
            icol_sg = icols[0]
            icol_w = sum(nidxs) // 16
            gt = gpool.tile([P, max_sgc, D], bf16, tag="gather")
            it = ipool.tile([P, icol_w], i16, tag="gidx")
            nc.sync.dma_start(
                out=it[:],
                in_=idx_in[:, col_base[g] + icol_sg :
                           col_base[g] + icol_sg + icol_w])
            oet = ipool.tile([P, 2 * sg_chunks], f32, tag="gonse")
            nc.sync.dma_start(
                out=oet[:],
                in_=onse_in[:, chk_base[g] + 2 * c0_sg :
                            chk_base[g] + 2 * (c0_sg + sg_chunks)])
            ot = oet[:, :sg_chunks]
            et = oet[:, sg_chunks:]
            for b in range(NBANK):
                nch = nchs[b]
                if nch == 0:
                    continue
                lo = c0s[b] - c0_sg
                ilo = icols[b] - icol_sg
                nidx = nidxs[b]
                bank_rows = min(cfg.bank_rows, npad - b * cfg.bank_rows)
                nc.gpsimd.dma_gather(
                    out_ap=gt[:, lo : lo + nch, :],
                    in_ap=featsr[b * cfg.bank_rows :
                                 b * cfg.bank_rows + bank_rows, :],
                    idxs_ap=it[:, ilo : ilo + nidx // 16],
                    num_idxs=nidx, num_idxs_reg=nidx,
                    elem_size=D, single_packet=False)
            nquad = (kn + 3) // 4
            psTs = [ppool.tile([P, 4 * D], f32, tag="psT", name="psT")
                    for _ in range(nquad)]
            spans = sch["spans"][si]
            oh_cache = {}
            for (b, j, ki, st, sp) in sch["mms"][si]:
                lo = c0s[b] - c0_sg + j
                pv = min(P, nidxs[b] - j * P)  # valid rows in this chunk
                klo, khi = spans.get((b, j), (ki, ki))
                if ki < klo or ki > khi:
                    # dummy zero matmul for an uncovered slot: one-off
                    # narrow one-hot (off values there never match ki)
                    oh = ohpool.tile([P, D], bf16, tag="ohw", name="ohw")
                    nc.vector.tensor_scalar(
                        out=oh[:], in0=ramp_sb[:, ki * P : (ki + 1) * P],
                        scalar1=ot[:, lo : lo + 1],
                        scalar2=et[:, lo : lo + 1],
                        op0=Alu.is_equal, op1=Alu.mult)
                    rhs = oh[:pv, :]
                else:
                    if (b, j) not in oh_cache:
                        span = khi - klo + 1
                        ohw = ohpool.tile([P, span * D], bf16, tag="ohw",
                                          name="ohw")
                        eng = nc.vector
                        if cfg.oh_gpsimd_mod and (
                                ecnt % cfg.oh_gpsimd_mod == 0):
                            eng = nc.gpsimd
                        ecnt += 1
                        eng.tensor_scalar(
                            out=ohw[:],
                            in0=ramp_sb[:, klo * P : (khi + 1) * P],
                            scalar1=ot[:, lo : lo + 1],
                            scalar2=et[:, lo : lo + 1],
                            op0=Alu.is_equal, op1=Alu.mult)
                        oh_cache[(b, j)] = ohw
                    ohw = oh_cache[(b, j)]
                    rhs = ohw[:pv, (ki - klo) * D : (ki - klo + 1) * D]
                q, r = divmod(ki, 4)
                nc.tensor.matmul(
                    out=psTs[q][:, r * D : (r + 1) * D],
                    lhsT=gt[:pv, lo, :], rhs=rhs, start=st, stop=sp)
            stg = spool.tile([P, kn * D], bf16, tag="stg")
            for q in range(nquad):
                kq = min(4, kn - 4 * q)
                aggsb = aggpool.tile([P, 4 * D], bf16, tag="aggsb")
                nc.scalar.activation(out=aggsb[:, : kq * D],
                                     in_=psTs[q][:, : kq * D],
                                     func=Act.Copy)
                hps = hpool.tile([P, 4 * D], f32)
                for r in range(kq):
                    nc.tensor.matmul(out=hps[:, r * D : (r + 1) * D],
                                     lhsT=aggsb[:, r * D : (r + 1) * D],
                                     rhs=w_sb[:], start=True, stop=True)
                ss = stg[:, 4 * q * D : (4 * q + kq) * D]
                if use_bias:
                    hb2 = tpool.tile([P, 4 * D], f32, tag="hb2")
                    nc.vector.tensor_tensor(out=hb2[:, : kq * D],
                                            in0=hps[:, : kq * D],
                                            in1=b_sb[:, : kq * D], op=Alu.add)
                    neg = tpool.tile([P, 4 * D], f32, tag="neg")
                    nc.vector.tensor_scalar(
                        out=neg[:, : kq * D], in0=hb2[:, : kq * D],
                        scalar1=0.0, scalar2=a_sb[:, :1],
                        op0=Alu.min, op1=Alu.mult)
                    pos = tpool.tile([P, 4 * D], f32, tag="pos")
                    nc.vector.tensor_scalar(
                        out=pos[:, : kq * D], in0=hb2[:, : kq * D],
                        scalar1=0.0, scalar2=None, op0=Alu.max)
                    nc.vector.tensor_tensor(out=ss, in0=neg[:, : kq * D],
                                            in1=pos[:, : kq * D], op=Alu.add)
                elif cfg.act_prelu:
                    nc.scalar.activation(
                        out=ss, in_=hps[:, : kq * D], func=Act.Prelu,
                        alpha=a_sb[:, :1])
                else:
                    neg = tpool.tile([P, 4 * D], f32, tag="neg")
                    nc.vector.tensor_scalar(
                        out=neg[:, : kq * D], in0=hps[:, : kq * D],
                        scalar1=0.0, scalar2=a_sb[:, :1],
                        op0=Alu.min, op1=Alu.mult)
                    pos = tpool.tile([P, 4 * D], f32, tag="pos")
                    nc.vector.tensor_scalar(
                        out=pos[:, : kq * D], in0=hps[:, : kq * D],
                        scalar1=0.0, scalar2=None, op0=Alu.max)
                    nc.vector.tensor_tensor(out=ss, in0=neg[:, : kq * D],
                                            in1=pos[:, : kq * D], op=Alu.add)
            nc.sync.dma_start(out=out[g, si, :, : kn * D], in_=stg[:])
    return out


# --------------------------------------------------------------------------
# Driver
# --------------------------------------------------------------------------
def _build_program(cfg: Config, meta):
    import concourse.bacc as bacc
    import concourse.tile as tile

    nc = bacc.Bacc("TRN2", target_bir_lowering=False, debug=False,
                   enable_asserts=False, num_devices=cfg.n_cores)
    with tile.TileContext(nc) as tc:
        build_kernel(nc, tc, cfg, meta)
    nc.compile()
    return nc


def _unscramble(results, plans, meta, cfg: Config):
    n = cfg.n_nodes
    full = np.zeros((2, n, D), np.float32)
    for g in range(2):
        sched = meta["scheds"][g]
        ct_all = plans[g]["core_tiles"]
        for core in range(cfg.n_cores):
            oc = np.asarray(results[core]["out"], dtype=np.float32)
            for si, (k0, kn) in enumerate(sched["sg_list"]):
                for ki in range(kn):
                    t = int(ct_all[core, k0 + ki])
                    if t < 0:
                        continue
                    r0 = t * P
                    r1 = min(r0 + P, n)
                    full[g, r0:r1] = oc[g, si, : r1 - r0,
                                        ki * D : (ki + 1) * D]
    return full


_PROGRAM_CACHE = {}


def _sched_key(sched):
    return (tuple(tuple(x) for x in sched["nch"]),
            tuple(mm for sgm in sched["mms"] for mm in sgm))


def run(inputs, cfg: Config, trace=False):
    from concourse.bass_utils import run_bass_kernel_spmd

    in_maps, plans, meta = preprocess(
        inputs["feats"], inputs["W"], inputs["b"], inputs["prelu_a"],
        inputs["src_pos"], inputs["dst_pos"],
        inputs["src_neg"], inputs["dst_neg"], cfg)

    key = (cfg.n_nodes, cfg.n_cores, cfg.sg,
           cfg.act_prelu, cfg.oh_gpsimd_mod, cfg.gbufs,
           _sched_key(meta["scheds"][0]), _sched_key(meta["scheds"][1]),
           meta["use_bias"])
    nc = _PROGRAM_CACHE.get(key)
    if nc is None:
        nc = _build_program(cfg, meta)
        _PROGRAM_CACHE[key] = nc

    kwargs = {}
    if trace:
        kwargs = dict(trace=True, tmpdir=tempfile.mkdtemp(prefix="bgc_trace_"))
    res = run_bass_kernel_spmd(nc, in_maps, core_ids=list(range(cfg.n_cores)),
                               **kwargs)
    full = _unscramble(res.results, plans, meta, cfg)
    return full, res


def kernel(**inputs) -> np.ndarray:
    cfg = Config()
    full, _ = run(inputs, cfg)
    return full


# revision 39
# speedup vs baseline: 1.0194x; 1.0008x over previous
"""Bass/Trainium2 kernel for BiGraphContrastLayer (GNN message passing).

Computes, for two edge lists (pos/neg) over the same node features:
    h_g = PReLU( D_in^-1/2 A_g D_out^-1/2 feats @ W + b )
returning stack([h_pos, h_neg]) of shape [2, N, Dout].

Strategy (8 NeuronCores, SPMD, no collectives), using the linearity
   (D_in^-1/2 A D_out^-1/2 feats) @ W = (D_in^-1/2 A D_out^-1/2 feats) W:

  No y-phase: dma_gather (int16 idx, 4 src-row banks of <=25088 rows)
  pulls RAW bf16 feats rows straight from the (host-cast, padded) input.
  Edges are bucketed by dst tile (slot), dealt to cores, and packed
  CONTINUOUSLY per (supergroup, bank) into 128-row chunks — a chunk may
  span several dst slots; per (chunk, slot) a weighted one-hot
  (rhs[p, j] = ns[src_p]*nd[dst_p] iff slot-relative dst position match,
  built on DVE via is_equal(int16 ramp slice, f32 off) + mult) matmul
  accumulates into the slot's quarter of a quad [128, 512] PSUM bank
  aggT[feat, dst]. Per quad: one cast copy aggT -> SBUF bf16, per slot a
  matmul (lhsT=aggT slice, rhs=W) -> h PSUM quad, one PReLU on ScalarE,
  bf16 store in a p-major layout (contiguous per partition).

  All 8 cores share one instruction stream: the chunk/matmul schedule is
  the UNION over cores (max chunk counts, union slot spans); cores
  lacking edges for a scheduled (chunk, slot) build an all-zero one-hot.

  Host does index/metadata work only: degree bincounts -> per-edge norm
  weights, sorting, bucketing, dealing, packing, int16 wrapped gather
  indices, replicating the small W/b/prelu params per the sharding hint.
"""

import math
import tempfile
from dataclasses import dataclass

import numpy as np

P = 128   # partitions
D = 128   # feature dim (Din == Dout == 128)
NBANK = 4
BF16 = np.dtype("bfloat16")
OFF_PAD = 4096.0  # off value matching no ramp slice


# --------------------------------------------------------------------------
# Config
# --------------------------------------------------------------------------
@dataclass
class Config:
    n_nodes: int = 100000
    n_cores: int = 8
    sg: int = 16       # dst-tile slots per supergroup (multiple of 4 best)
    oh_gpsimd_mod: int = 0    # every Nth one-hot build goes to GpSimd (0=off)
    act_prelu: bool = True    # final PReLU on ScalarE (not in sim)
    gbufs: int = 4            # gather buffer count
    ipbufs: int = 4           # idx buffer count
    ppbufs: int = 6           # PSUM quad accumulator banks
    hpbufs: int = 2           # PSUM h banks

    @property
    def t_global(self) -> int:
        return math.ceil(self.n_nodes / P)

    @property
    def n_pad(self) -> int:
        return self.t_global * P

    @property
    def t_core(self) -> int:
        return math.ceil(self.t_global / self.n_cores)

    @property
    def bank_tiles(self) -> int:
        return math.ceil(self.t_global / NBANK)

    @property
    def bank_rows(self) -> int:
        return self.bank_tiles * P

    @property
    def n_sg(self) -> int:
        return math.ceil(self.t_core / self.sg)


# --------------------------------------------------------------------------
# Host-side preprocessing (integer index / edge-weight metadata only)
# --------------------------------------------------------------------------
def _plan_graph(src, dst, nse_edge, cfg: Config):
    """Bucket edges by dst tile, sort by src within tile, bank-split, and
    deal tiles to cores (snake by total edge count for balance).

    Returns dict with:
      core_tiles  [n_cores, t_core]  global tile id per slot (-1 null)
      counts      [n_cores, t_core, NBANK] per-slot-bank edge counts
      tile_edges  list per global tile, per bank: (src_local, dstoff, nse)
    """
    tg, ncores, tcore = cfg.t_global, cfg.n_cores, cfg.t_core
    order = np.argsort(dst, kind="stable")
    src_s = src[order]
    dst_s = dst[order]
    nse_s = nse_edge[order]
    tile_cnt = np.bincount(dst_s // P, minlength=tg)
    starts = np.zeros(tg + 1, np.int64)
    np.cumsum(tile_cnt, out=starts[1:])

    bank_of = src_s // cfg.bank_rows
    tile_edges = []
    for t in range(tg):
        e0, e1 = int(starts[t]), int(starts[t + 1])
        per_bank = []
        for b in range(NBANK):
            m = bank_of[e0:e1] == b
            per_bank.append((
                (src_s[e0:e1][m] - b * cfg.bank_rows).astype(np.int64),
                (dst_s[e0:e1][m] % P).astype(np.int64),
                nse_s[e0:e1][m].astype(np.float32),
            ))
        tile_edges.append(per_bank)

    # Deal tiles by descending total edges; within each rank-group of
    # ncores tiles, greedily give each tile to the core whose running
    # per-bank supergroup sums stay smallest (bank-aware LPT) — the
    # shared per-(sg, bank) chunk count is the max over cores, so
    # minimizing the max per-bank deficit minimizes gather padding.
    bank_cnt = np.zeros((tg, NBANK), np.int64)
    for t in range(tg):
        for b in range(NBANK):
            bank_cnt[t, b] = len(tile_edges[t][b][0])
    keys = np.argsort(tile_cnt, kind="stable")[::-1]
    core_tiles = np.full((ncores, tcore), -1, np.int64)
    run = np.zeros((ncores, NBANK), np.int64)
    for k in range(tcore):
        if k % cfg.sg == 0:
            run[:] = 0  # new supergroup
        grp = keys[k * ncores : (k + 1) * ncores]
        free = list(range(ncores))
        for t in grp:
            proj = run[free] + bank_cnt[t]
            i = int(np.argmin(proj.max(axis=1) + 1e-3 * proj.sum(axis=1)))
            c = free.pop(i)
            core_tiles[c, k] = t
            run[c] += bank_cnt[t]

    # Refinement: within each supergroup, greedily swap same-slot tile
    # assignments between cores while it reduces sum_b max_c of the
    # per-bank supergroup sums (the quantity the shared gather pays for).
    def _cnt(t):
        return bank_cnt[t] if t >= 0 else np.zeros(NBANK, np.int64)

    for k0 in range(0, tcore, cfg.sg):
        kn = min(cfg.sg, tcore - k0)
        S = np.zeros((ncores, NBANK), np.int64)
        for c in range(ncores):
            for k in range(k0, k0 + kn):
                S[c] += _cnt(core_tiles[c, k])
        for _ in range(4):
            improved = False
            base = S.max(axis=0).sum()
            for k in range(k0, k0 + kn):
                for c1 in range(ncores):
                    for c2 in range(c1 + 1, ncores):
                        d1 = _cnt(core_tiles[c2, k]) - _cnt(core_tiles[c1, k])
                        S[c1] += d1
                        S[c2] -= d1
                        new = S.max(axis=0).sum()
                        if new < base:
                            base = new
                            core_tiles[c1, k], core_tiles[c2, k] = (
                                core_tiles[c2, k], core_tiles[c1, k])
                            improved = True
                        else:
                            S[c1] -= d1
                            S[c2] += d1
            if not improved:
                break

    counts = np.zeros((ncores, tcore, NBANK), np.int64)
    for c in range(ncores):
        for k in range(tcore):
            t = core_tiles[c, k]
            if t < 0:
                continue
            for b in range(NBANK):
                counts[c, k, b] = len(tile_edges[t][b][0])
    return dict(core_tiles=core_tiles, counts=counts, tile_edges=tile_edges)


def _schedule_graph(plan, cfg: Config):
    """Shared (all-cores) chunk layout + matmul schedule for one graph.

    Chunks are packed continuously per (supergroup, bank): each core lays
    its slots' bank-b edges end-to-end; the shared chunk count is the max
    over cores, the per-chunk slot list is the union over cores.

    Returns dict:
      sg_list  [(k0, kn)]
      nch      [n_sg][NBANK] shared chunk counts
      c0       [n_sg][NBANK] global first-chunk index
      mms      [n_sg] ordered list of (bank, j, slot_local, start, stop)
      n_chunks total
    """
    counts = plan["counts"]
    ncores, tcore = cfg.n_cores, cfg.t_core
    # Supergroups: full-size bodies, but split the tail into small sgs so
    # the end-of-kernel drain (matmul/copy/W/prelu/store of the last sg)
    # is short and overlaps the final gathers.
    sg_list = []
    k0 = 0
    while k0 < tcore:
        rem = tcore - k0
        if rem > cfg.sg + cfg.sg // 2:
            kn = cfg.sg
        elif rem > cfg.sg:
            kn = (rem + 2) // 3  # three medium tail sgs
        elif rem > 4:
            kn = (rem + 1) // 2  # two small tail sgs
        else:
            kn = rem
        sg_list.append((k0, kn))
        k0 += kn

    nch_all, c0_all, mms_all, spans_all, nidx_all = [], [], [], [], []
    cpos = 0
    for (k0, kn) in sg_list:
        nch_sg = []
        c0_sg = []
        nidx_sg = []
        touches = []  # (slot, bank, j) -> sorted slot-major for PSUM groups
        covered = set()
        for b in range(NBANK):
            cum = np.zeros((ncores, kn + 1), np.int64)
            np.cumsum(counts[:, k0 : k0 + kn, b], axis=1, out=cum[:, 1:])
            maxn = int(max(cum[c, kn] for c in range(ncores)))
            nidx = -(-maxn // 16) * 16  # descriptors charged = num_idxs
            nch = -(-nidx // P)
            nch_sg.append(nch)
            nidx_sg.append(nidx)
            c0_sg.append(cpos)
            cpos += nch
            for j in range(nch):
                lo, hi = j * P, (j + 1) * P
                slots = set()
                for c in range(ncores):
                    for ki in range(kn):
                        if cum[c, ki] < hi and cum[c, ki + 1] > lo:
                            slots.add(ki)
                for ki in sorted(slots):
                    touches.append((ki, b, j))
                    covered.add(ki)
        # slot-major order: each slot's PSUM accumulation group closes
        # before the next one opens in the same PSUM bank
        entries = [(b, j, ki) for (ki, b, j) in sorted(touches)]
        # per-chunk slot span (for the wide one-hot build)
        spans = {}
        for (ki, b, j) in touches:
            lo, hi = spans.get((b, j), (ki, ki))
            spans[(b, j)] = (min(lo, ki), max(hi, ki))
        if sum(nch_sg) == 0:
            # fully empty supergroup: force one pad chunk in bank 0
            nch_sg[0] = 1
            nidx_sg[0] = 16
            for b in range(1, NBANK):
                c0_sg[b] = c0_sg[0] + 1
            cpos += 1
        # zero-coverage slots get one all-zero matmul on the sg's first
        # populated bank's chunk 0 (no core has a matching off there, by
        # construction)
        dummy_bank = next(b for b in range(NBANK) if nch_sg[b] > 0)
        for ki in range(kn):
            if ki not in covered:
                entries.append((dummy_bank, 0, ki))
        # start/stop per slot over entry order
        first, last = {}, {}
        for i, (b, j, ki) in enumerate(entries):
            if ki not in first:
                first[ki] = i
            last[ki] = i
        mms = [(b, j, ki, i == first[ki], i == last[ki])
               for i, (b, j, ki) in enumerate(entries)]
        nch_all.append(nch_sg)
        c0_all.append(c0_sg)
        mms_all.append(mms)
        spans_all.append(spans)
        nidx_all.append(nidx_sg)
    # idx column layout: per (sg, bank) call, nidx/16 int16 columns
    icol_all = []
    icol = 0
    for si in range(len(sg_list)):
        icol_sg = []
        for b in range(NBANK):
            icol_sg.append(icol)
            icol += nidx_all[si][b] // 16
        icol_all.append(icol_sg)
    return dict(sg_list=sg_list, nch=nch_all, c0=c0_all, mms=mms_all,
                spans=spans_all, nidx=nidx_all, icol=icol_all,
                n_chunks=cpos, ncols=icol)


def _fill_core_graph(plan, sched, core, cfg: Config):
    """Build IDX16 (wrapped), OFF and NSE arrays for one core, one graph."""
    n_chunks = sched["n_chunks"]
    idx = np.zeros((n_chunks, P), np.int16)
    off = np.full((n_chunks, P), OFF_PAD, np.float32)
    nse = np.zeros((n_chunks, P), np.float32)
    core_tiles = plan["core_tiles"]
    tile_edges = plan["tile_edges"]
    for si, (k0, kn) in enumerate(sched["sg_list"]):
        for b in range(NBANK):
            nch = sched["nch"][si][b]
            if nch == 0:
                continue
            c0 = sched["c0"][si][b]
            es = np.zeros(nch * P, np.int64)
            eo = np.full(nch * P, OFF_PAD, np.float32)
            en = np.zeros(nch * P, np.float32)
            pos = 0
            for ki in range(kn):
                t = core_tiles[core, k0 + ki]
                if t < 0:
                    continue
                s_rows, s_off, s_nse = tile_edges[t][b]
                nb = len(s_rows)
                es[pos : pos + nb] = s_rows
                eo[pos : pos + nb] = ki * P + s_off
                en[pos : pos + nb] = s_nse
                pos += nb
            idx[c0 : c0 + nch] = es.reshape(nch, P)
            off[c0 : c0 + nch] = eo.reshape(nch, P)
            nse[c0 : c0 + nch] = en.reshape(nch, P)
    # wrap: flat slot i (within a call's first num_idxs slots) ->
    # [i%16, i//16], replicated to 128 partitions. Only num_idxs (<=
    # nch*128, 16-aligned) indices are shipped/charged per call; the
    # remaining tail of the last chunk is never gathered and its off
    # stays OFF_PAD.
    idx_w = np.zeros((P, sched["ncols"]), np.int16)
    for si in range(len(sched["sg_list"])):
        for b in range(NBANK):
            nidx = sched["nidx"][si][b]
            if nidx == 0:
                continue
            c0 = sched["c0"][si][b]
            icol = sched["icol"][si][b]
            flat = idx[c0 : c0 + sched["nch"][si][b]].reshape(-1)[:nidx]
            blk = flat.reshape(-1, 16).T  # [16, nidx/16]
            idx_w[:, icol : icol + nidx // 16] = np.tile(blk, (8, 1))
    # interleave off/nse per supergroup block ([off_cols | nse_cols]) so
    # the per-sg DMA load is one >=512B-per-partition transfer (no 2x
    # small-descriptor penalty)
    offT, nseT = off.T, nse.T
    onse = np.empty((P, 2 * n_chunks), np.float32)
    for si in range(len(sched["sg_list"])):
        nchs = sched["nch"][si]
        c0_sg = min(sched["c0"][si][b] for b in range(NBANK) if nchs[b] > 0)
        sgc = sum(nchs)
        onse[:, 2 * c0_sg : 2 * c0_sg + sgc] = offT[:, c0_sg : c0_sg + sgc]
        onse[:, 2 * c0_sg + sgc : 2 * (c0_sg + sgc)] = (
            nseT[:, c0_sg : c0_sg + sgc])
    return idx_w, onse  # [P, 2*n_chunks]


def preprocess(feats, W, b, prelu_a, src_pos, dst_pos, src_neg, dst_neg,
               cfg: Config):
    n, ncores = cfg.n_nodes, cfg.n_cores
    feats = np.asarray(feats, np.float32)
    W = np.asarray(W, np.float32)
    b = np.asarray(b, np.float32)
    prelu_a = np.asarray(prelu_a, np.float32)

    featsr = np.zeros((cfg.n_pad, D), BF16)  # row-major, padded, bf16
    featsr[:n] = feats.astype(BF16)

    plans, scheds = [], []
    for src, dst in ((src_pos, dst_pos), (src_neg, dst_neg)):
        src = np.asarray(src, np.int64)
        dst = np.asarray(dst, np.int64)
        deg_out = np.bincount(src, minlength=n).astype(np.float32)
        deg_in = np.bincount(dst, minlength=n).astype(np.float32)
        ns = np.where(deg_out > 0, 1.0 / np.sqrt(np.maximum(deg_out, 1.0)),
                      0.0).astype(np.float32)
        nd = np.where(deg_in > 0, 1.0 / np.sqrt(np.maximum(deg_in, 1.0)),
                      0.0).astype(np.float32)
        nse_edge = ns[src] * nd[dst]
        plan = _plan_graph(src, dst, nse_edge, cfg)
        plans.append(plan)
        scheds.append(_schedule_graph(plan, cfg))

    ramp = np.tile(np.arange(cfg.sg * P, dtype=np.int16), (P, 1))
    a_rep = np.full((P, 1), float(prelu_a.reshape(-1)[0]), np.float32)
    b_rep = np.tile(b.reshape(1, D), (P, 4)).astype(np.float32)

    in_maps = []
    for core in range(ncores):
        iw_p, onse_p = _fill_core_graph(plans[0], scheds[0], core, cfg)
        iw_n, onse_n = _fill_core_graph(plans[1], scheds[1], core, cfg)
        in_maps.append({
            "featsr": featsr,
            "w_in": W,
            "a_rep": a_rep,
            "b_rep": b_rep,
            "idx_in": np.concatenate([iw_p, iw_n], axis=1),
            "onse_in": np.concatenate([onse_p, onse_n], axis=1),
            "ramp_in": ramp,
        })
    meta = {
        "scheds": scheds,
        "use_bias": bool(np.any(b != 0.0)),
    }
    return in_maps, plans, meta


# --------------------------------------------------------------------------
# Device kernel builder
# --------------------------------------------------------------------------
def build_kernel(nc, tc, cfg: Config, meta):
    from contextlib import ExitStack

    import concourse.mybir as mybir

    f32 = mybir.dt.float32
    bf16 = mybir.dt.bfloat16
    i16 = mybir.dt.int16
    Alu = mybir.AluOpType
    Act = mybir.ActivationFunctionType

    npad = cfg.n_pad
    scheds = meta["scheds"]
    use_bias = meta["use_bias"]
    n_chunks = [scheds[g]["n_chunks"] for g in range(2)]
    ncols = [scheds[g]["ncols"] for g in range(2)]
    n_sg = len(scheds[0]["sg_list"])
    assert len(scheds[1]["sg_list"]) == n_sg

    featsr = nc.dram_tensor("featsr", [npad, D], bf16,
                            kind="ExternalInput").ap()
    w_in = nc.dram_tensor("w_in", [P, D], f32, kind="ExternalInput").ap()
    a_rep = nc.dram_tensor("a_rep", [P, 1], f32, kind="ExternalInput").ap()
    b_rep = nc.dram_tensor("b_rep", [P, 4 * D], f32, kind="ExternalInput").ap()
    idx_in = nc.dram_tensor("idx_in", [P, sum(ncols)], i16,
                            kind="ExternalInput").ap()
    onse_in = nc.dram_tensor("onse_in", [P, 2 * sum(n_chunks)], f32,
                             kind="ExternalInput").ap()
    ramp_in = nc.dram_tensor("ramp_in", [P, cfg.sg * P], i16,
                             kind="ExternalInput").ap()
    out = nc.dram_tensor("out", [2, n_sg, P, cfg.sg * D], bf16,
                         kind="ExternalOutput").ap()

    with ExitStack() as ctx:
        const = ctx.enter_context(tc.tile_pool(name="const", bufs=1))
        gpool = ctx.enter_context(tc.tile_pool(name="gpool", bufs=cfg.gbufs))
        ipool = ctx.enter_context(tc.tile_pool(name="ipool", bufs=cfg.ipbufs))
        ohpool = ctx.enter_context(tc.tile_pool(name="ohpool", bufs=24))
        aggpool = ctx.enter_context(tc.tile_pool(name="aggpool", bufs=4))
        tpool = ctx.enter_context(tc.tile_pool(name="tpool", bufs=4))
        spool = ctx.enter_context(tc.tile_pool(name="spool", bufs=3))
        ppool = ctx.enter_context(tc.tile_pool(name="ppool", bufs=cfg.ppbufs,
                                               space="PSUM"))
        hpool = ctx.enter_context(tc.tile_pool(name="hpool", bufs=cfg.hpbufs,
                                               space="PSUM"))

        # ---- constants ----
        w_sb = const.tile([P, D], bf16)
        nc.gpsimd.dma_start(out=w_sb[:], in_=w_in)  # f32 -> bf16 cast DMA
        ramp_sb = const.tile([P, cfg.sg * P], i16)
        nc.gpsimd.dma_start(out=ramp_sb[:], in_=ramp_in)
        a_sb = const.tile([P, 1], f32)
        nc.gpsimd.dma_start(out=a_sb[:], in_=a_rep)
        if use_bias:
            b_sb = const.tile([P, 4 * D], f32)
            nc.sync.dma_start(out=b_sb[:], in_=b_rep)

        max_sgc = max(sum(scheds[g]["nch"][si]) for g in range(2)
                      for si in range(n_sg))

        # ---- gather + weighted one-hot segment-sum + @W + prelu ----
        col_base = [0, ncols[0]]          # idx column offset per graph
        chk_base = [0, 2 * n_chunks[0]]   # onse column offset per graph
        # interleave the two graphs' supergroups so one graph's gathers fill
        # DMA while the other's PSUM chain drains
        jobs = []
        for si in range(n_sg):
            for g in range(2):
                jobs.append((g, si))
        ecnt = 0
        for (g, si) in jobs:
            sch = scheds[g]
            (k0, kn) = sch["sg_list"][si]
            nchs = sch["nch"][si]
            c0s = sch["c0"][si]
            c0_sg = min(c0s[b] for b in range(NBANK) if nchs[b] > 0)
            sg_chunks = sum(nchs)
            nidxs = sch["nidx"][si]
            icols = sch["icol"][si]
            icol_sg = icols[0]
            icol_w = sum(nidxs) // 16
            gt = gpool.tile([P, max_sgc, D], bf16, tag="gather")
            it = ipool.tile([P, icol_w], i16, tag="gidx")
            nc.sync.dma_start(
                out=it[:],
                in_=idx_in[:, col_base[g] + icol_sg :
                           col_base[g] + icol_sg + icol_w])
            oet = ipool.tile([P, 2 * sg_chunks], f32, tag="gonse")
            nc.sync.dma_start(
                out=oet[:],
                in_=onse_in[:, chk_base[g] + 2 * c0_sg :
                            chk_base[g] + 2 * (c0_sg + sg_chunks)])
            ot = oet[:, :sg_chunks]
            et = oet[:, sg_chunks:]
            for b in range(NBANK):
                nch = nchs[b]
                if nch == 0:
                    continue
                lo = c0s[b] - c0_sg
                ilo = icols[b] - icol_sg
                nidx = nidxs[b]
                bank_rows = min(cfg.bank_rows, npad - b * cfg.bank_rows)
                nc.gpsimd.dma_gather(
                    out_ap=gt[:, lo : lo + nch, :],
                    in_ap=featsr[b * cfg.bank_rows :
                                 b * cfg.bank_rows + bank_rows, :],
                    idxs_ap=it[:, ilo : ilo + nidx // 16],
                    num_idxs=nidx, num_idxs_reg=nidx,
                    elem_size=D, single_packet=False)
            nquad = (kn + 3) // 4
            psTs = [ppool.tile([P, 4 * D], f32, tag="psT", name="psT")
                    for _ in range(nquad)]
            spans = sch["spans"][si]
            oh_cache = {}
            for (b, j, ki, st, sp) in sch["mms"][si]:
                lo = c0s[b] - c0_sg + j
                pv = min(P, nidxs[b] - j * P)  # valid rows in this chunk
                klo, khi = spans.get((b, j), (ki, ki))
                if ki < klo or ki > khi:
                    # dummy zero matmul for an uncovered slot: one-off
                    # narrow one-hot (off values there never match ki)
                    oh = ohpool.tile([P, D], bf16, tag="ohw", name="ohw")
                    nc.vector.tensor_scalar(
                        out=oh[:], in0=ramp_sb[:, ki * P : (ki + 1) * P],
                        scalar1=ot[:, lo : lo + 1],
                        scalar2=et[:, lo : lo + 1],
                        op0=Alu.is_equal, op1=Alu.mult)
                    rhs = oh[:pv, :]
                else:
                    if (b, j) not in oh_cache:
                        span = khi - klo + 1
                        ohw = ohpool.tile([P, span * D], bf16, tag="ohw",
                                          name="ohw")
                        eng = nc.vector
                        if cfg.oh_gpsimd_mod and (
                                ecnt % cfg.oh_gpsimd_mod == 0):
                            eng = nc.gpsimd
                        ecnt += 1
                        eng.tensor_scalar(
                            out=ohw[:],
                            in0=ramp_sb[:, klo * P : (khi + 1) * P],
                            scalar1=ot[:, lo : lo + 1],
                            scalar2=et[:, lo : lo + 1],
                            op0=Alu.is_equal, op1=Alu.mult)
                        oh_cache[(b, j)] = ohw
                    ohw = oh_cache[(b, j)]
                    rhs = ohw[:pv, (ki - klo) * D : (ki - klo + 1) * D]
                q, r = divmod(ki, 4)
                nc.tensor.matmul(
                    out=psTs[q][:, r * D : (r + 1) * D],
                    lhsT=gt[:pv, lo, :], rhs=rhs, start=st, stop=sp)
            stg = spool.tile([P, kn * D], bf16, tag="stg")
            for q in range(nquad):
                kq = min(4, kn - 4 * q)
                aggsb = aggpool.tile([P, 4 * D], bf16, tag="aggsb")
                nc.scalar.activation(out=aggsb[:, : kq * D],
                                     in_=psTs[q][:, : kq * D],
                                     func=Act.Copy)
                hps = hpool.tile([P, 4 * D], f32)
                for r in range(kq):
                    nc.tensor.matmul(out=hps[:, r * D : (r + 1) * D],
                                     lhsT=aggsb[:, r * D : (r + 1) * D],
                                     rhs=w_sb[:], start=True, stop=True)
                ss = stg[:, 4 * q * D : (4 * q + kq) * D]
                if use_bias:
                    hb2 = tpool.tile([P, 4 * D], f32, tag="hb2")
                    nc.vector.tensor_tensor(out=hb2[:, : kq * D],
                                            in0=hps[:, : kq * D],
                                            in1=b_sb[:, : kq * D], op=Alu.add)
                    neg = tpool.tile([P, 4 * D], f32, tag="neg")
                    nc.vector.tensor_scalar(
                        out=neg[:, : kq * D], in0=hb2[:, : kq * D],
                        scalar1=0.0, scalar2=a_sb[:, :1],
                        op0=Alu.min, op1=Alu.mult)
                    pos = tpool.tile([P, 4 * D], f32, tag="pos")
                    nc.vector.tensor_scalar(
                        out=pos[:, : kq * D], in0=hb2[:, : kq * D],
                        scalar1=0.0, scalar2=None, op0=Alu.max)
                    nc.vector.tensor_tensor(out=ss, in0=neg[:, : kq * D],
                                            in1=pos[:, : kq * D], op=Alu.add)
                elif cfg.act_prelu:
                    nc.scalar.activation(
                        out=ss, in_=hps[:, : kq * D], func=Act.Prelu,
                        alpha=a_sb[:, :1])
                else:
                    neg = tpool.tile([P, 4 * D], f32, tag="neg")
                    nc.vector.tensor_scalar(
                        out=neg[:, : kq * D], in0=hps[:, : kq * D],
                        scalar1=0.0, scalar2=a_sb[:, :1],
                        op0=Alu.min, op1=Alu.mult)
                    pos = tpool.tile([P, 4 * D], f32, tag="pos")
                    nc.vector.tensor_scalar(
                        out=pos[:, : kq * D], in0=hps[:, : kq * D],
                        scalar1=0.0, scalar2=None, op0=Alu.max)
                    nc.vector.tensor_tensor(out=ss, in0=neg[:, : kq * D],
                                            in1=pos[:, : kq * D], op=Alu.add)
            nc.sync.dma_start(out=out[g, si, :, : kn * D], in_=stg[:])
    return out


# --------------------------------------------------------------------------
# Driver
# --------------------------------------------------------------------------
def _build_program(cfg: Config, meta):
    import concourse.bacc as bacc
    import concourse.tile as tile

    nc = bacc.Bacc("TRN2", target_bir_lowering=False, debug=False,
                   enable_asserts=False, num_devices=cfg.n_cores)
    with tile.TileContext(nc) as tc:
        build_kernel(nc, tc, cfg, meta)
    nc.compile()
    return nc


def _unscramble(results, plans, meta, cfg: Config):
    n = cfg.n_nodes
    full = np.zeros((2, n, D), np.float32)
    for g in range(2):
        sched = meta["scheds"][g]
        ct_all = plans[g]["core_tiles"]
        for core in range(cfg.n_cores):
            oc = np.asarray(results[core]["out"], dtype=np.float32)
            for si, (k0, kn) in enumerate(sched["sg_list"]):
                for ki in range(kn):
                    t = int(ct_all[core, k0 + ki])
                    if t < 0:
                        continue
                    r0 = t * P
                    r1 = min(r0 + P, n)
                    full[g, r0:r1] = oc[g, si, : r1 - r0,
                                        ki * D : (ki + 1) * D]
    return full


_PROGRAM_CACHE = {}


def _sched_key(sched):
    return (tuple(tuple(x) for x in sched["nch"]),
            tuple(mm for sgm in sched["mms"] for mm in sgm))


def run(inputs, cfg: Config, trace=False):
    from concourse.bass_utils import run_bass_kernel_spmd

    in_maps, plans, meta = preprocess(
        inputs["feats"], inputs["W"], inputs["b"], inputs["prelu_a"],
        inputs["src_pos"], inputs["dst_pos"],
        inputs["src_neg"], inputs["dst_neg"], cfg)

    key = (cfg.n_nodes, cfg.n_cores, cfg.sg,
           cfg.act_prelu, cfg.oh_gpsimd_mod, cfg.gbufs,
           _sched_key(meta["scheds"][0]), _sched_key(meta["scheds"][1]),
           meta["use_bias"])
    nc = _PROGRAM_CACHE.get(key)
    if nc is None:
        nc = _build_program(cfg, meta)
        _PROGRAM_CACHE[key] = nc

    kwargs = {}
    if trace:
        kwargs = dict(trace=True, tmpdir=tempfile.mkdtemp(prefix="bgc_trace_"))
    res = run_bass_kernel_spmd(nc, in_maps, core_ids=list(range(cfg.n_cores)),
                               **kwargs)
    full = _unscramble(res.results, plans, meta, cfg)
    return full, res


def kernel(**inputs) -> np.ndarray:
    cfg = Config()
    full, _ = run(inputs, cfg)
    return full


# revision 43
# speedup vs baseline: 1.0256x; 1.0061x over previous
"""Bass/Trainium2 kernel for BiGraphContrastLayer (GNN message passing).

Computes, for two edge lists (pos/neg) over the same node features:
    h_g = PReLU( D_in^-1/2 A_g D_out^-1/2 feats @ W + b )
returning stack([h_pos, h_neg]) of shape [2, N, Dout].

Strategy (8 NeuronCores, SPMD, no collectives), using the linearity
   (D_in^-1/2 A D_out^-1/2 feats) @ W = (D_in^-1/2 A D_out^-1/2 feats) W:

  No y-phase: dma_gather (int16 idx, 4 src-row banks of <=25088 rows)
  pulls RAW bf16 feats rows straight from the (host-cast, padded) input.
  Edges are bucketed by dst tile (slot), dealt to cores, and packed
  CONTINUOUSLY per (supergroup, bank) into 128-row chunks — a chunk may
  span several dst slots; per (chunk, slot) a weighted one-hot
  (rhs[p, j] = ns[src_p]*nd[dst_p] iff slot-relative dst position match,
  built on DVE via is_equal(int16 ramp slice, f32 off) + mult) matmul
  accumulates into the slot's quarter of a quad [128, 512] PSUM bank
  aggT[feat, dst]. Per quad: one cast copy aggT -> SBUF bf16, per slot a
  matmul (lhsT=aggT slice, rhs=W) -> h PSUM quad, one PReLU on ScalarE,
  bf16 store in a p-major layout (contiguous per partition).

  All 8 cores share one instruction stream: the chunk/matmul schedule is
  the UNION over cores (max chunk counts, union slot spans); cores
  lacking edges for a scheduled (chunk, slot) build an all-zero one-hot.

  Host does index/metadata work only: degree bincounts -> per-edge norm
  weights, sorting, bucketing, dealing, packing, int16 wrapped gather
  indices, replicating the small W/b/prelu params per the sharding hint.
"""

import math
import tempfile
from dataclasses import dataclass

import numpy as np

P = 128   # partitions
D = 128   # feature dim (Din == Dout == 128)
NBANK = 4
BF16 = np.dtype("bfloat16")
OFF_PAD = 4096.0  # off value matching no ramp slice


# --------------------------------------------------------------------------
# Config
# --------------------------------------------------------------------------
@dataclass
class Config:
    n_nodes: int = 100000
    n_cores: int = 8
    sg: int = 16       # dst-tile slots per supergroup (multiple of 4 best)
    oh_gpsimd_mod: int = 0    # every Nth one-hot build goes to GpSimd (0=off)
    act_prelu: bool = True    # final PReLU on ScalarE (not in sim)
    gbufs: int = 4            # gather buffer count
    ipbufs: int = 4           # idx buffer count
    ppbufs: int = 6           # PSUM quad accumulator banks
    hpbufs: int = 2           # PSUM h banks

    @property
    def t_global(self) -> int:
        return math.ceil(self.n_nodes / P)

    @property
    def n_pad(self) -> int:
        return self.t_global * P

    @property
    def t_core(self) -> int:
        return math.ceil(self.t_global / self.n_cores)

    @property
    def bank_tiles(self) -> int:
        return math.ceil(self.t_global / NBANK)

    @property
    def bank_rows(self) -> int:
        return self.bank_tiles * P

    @property
    def n_sg(self) -> int:
        return math.ceil(self.t_core / self.sg)


# --------------------------------------------------------------------------
# Host-side preprocessing (integer index / edge-weight metadata only)
# --------------------------------------------------------------------------
def _plan_graph(src, dst, cfg: Config):
    """Bucket edges by dst tile, sort by src within tile, bank-split, and
    deal tiles to cores (snake by total edge count for balance).

    Returns dict with:
      core_tiles  [n_cores, t_core]  global tile id per slot (-1 null)
      counts      [n_cores, t_core, NBANK] per-slot-bank edge counts
      tile_edges  list per global tile, per bank: (src_local, dstoff, nse)
    """
    tg, ncores, tcore = cfg.t_global, cfg.n_cores, cfg.t_core
    order = np.argsort(dst, kind="stable")
    src_s = src[order]
    dst_s = dst[order]
    tile_cnt = np.bincount(dst_s // P, minlength=tg)
    starts = np.zeros(tg + 1, np.int64)
    np.cumsum(tile_cnt, out=starts[1:])

    bank_of = src_s // cfg.bank_rows
    tile_edges = []
    for t in range(tg):
        e0, e1 = int(starts[t]), int(starts[t + 1])
        per_bank = []
        for b in range(NBANK):
            m = bank_of[e0:e1] == b
            per_bank.append((
                (src_s[e0:e1][m] - b * cfg.bank_rows).astype(np.int64),
                (dst_s[e0:e1][m] % P).astype(np.int64),
            ))
        tile_edges.append(per_bank)

    # Deal tiles by descending total edges; within each rank-group of
    # ncores tiles, greedily give each tile to the core whose running
    # per-bank supergroup sums stay smallest (bank-aware LPT) — the
    # shared per-(sg, bank) chunk count is the max over cores, so
    # minimizing the max per-bank deficit minimizes gather padding.
    bank_cnt = np.zeros((tg, NBANK), np.int64)
    for t in range(tg):
        for b in range(NBANK):
            bank_cnt[t, b] = len(tile_edges[t][b][0])
    keys = np.argsort(tile_cnt, kind="stable")[::-1]
    core_tiles = np.full((ncores, tcore), -1, np.int64)
    run = np.zeros((ncores, NBANK), np.int64)
    for k in range(tcore):
        if k % cfg.sg == 0:
            run[:] = 0  # new supergroup
        grp = keys[k * ncores : (k + 1) * ncores]
        free = list(range(ncores))
        for t in grp:
            proj = run[free] + bank_cnt[t]
            i = int(np.argmin(proj.max(axis=1) + 1e-3 * proj.sum(axis=1)))
            c = free.pop(i)
            core_tiles[c, k] = t
            run[c] += bank_cnt[t]

    # Refinement: within each supergroup, greedily swap same-slot tile
    # assignments between cores while it reduces sum_b max_c of the
    # per-bank supergroup sums (the quantity the shared gather pays for).
    def _cnt(t):
        return bank_cnt[t] if t >= 0 else np.zeros(NBANK, np.int64)

    for k0 in range(0, tcore, cfg.sg):
        kn = min(cfg.sg, tcore - k0)
        S = np.zeros((ncores, NBANK), np.int64)
        for c in range(ncores):
            for k in range(k0, k0 + kn):
                S[c] += _cnt(core_tiles[c, k])
        for _ in range(4):
            improved = False
            base = S.max(axis=0).sum()
            for k in range(k0, k0 + kn):
                for c1 in range(ncores):
                    for c2 in range(c1 + 1, ncores):
                        d1 = _cnt(core_tiles[c2, k]) - _cnt(core_tiles[c1, k])
                        S[c1] += d1
                        S[c2] -= d1
                        new = S.max(axis=0).sum()
                        if new < base:
                            base = new
                            core_tiles[c1, k], core_tiles[c2, k] = (
                                core_tiles[c2, k], core_tiles[c1, k])
                            improved = True
                        else:
                            S[c1] -= d1
                            S[c2] += d1
            if not improved:
                break

    counts = np.zeros((ncores, tcore, NBANK), np.int64)
    for c in range(ncores):
        for k in range(tcore):
            t = core_tiles[c, k]
            if t < 0:
                continue
            for b in range(NBANK):
                counts[c, k, b] = len(tile_edges[t][b][0])
    return dict(core_tiles=core_tiles, counts=counts, tile_edges=tile_edges)


def _schedule_graph(plan, cfg: Config):
    """Shared (all-cores) chunk layout + matmul schedule for one graph.

    Chunks are packed continuously per (supergroup, bank): each core lays
    its slots' bank-b edges end-to-end; the shared chunk count is the max
    over cores, the per-chunk slot list is the union over cores.

    Returns dict:
      sg_list  [(k0, kn)]
      nch      [n_sg][NBANK] shared chunk counts
      c0       [n_sg][NBANK] global first-chunk index
      mms      [n_sg] ordered list of (bank, j, slot_local, start, stop)
      n_chunks total
    """
    counts = plan["counts"]
    ncores, tcore = cfg.n_cores, cfg.t_core
    # Supergroups: full-size bodies, but split the tail into small sgs so
    # the end-of-kernel drain (matmul/copy/W/prelu/store of the last sg)
    # is short and overlaps the final gathers.
    sg_list = []
    k0 = 0
    while k0 < tcore:
        rem = tcore - k0
        if rem > cfg.sg + cfg.sg // 2:
            kn = cfg.sg
        elif rem > cfg.sg:
            kn = (rem + 2) // 3  # three medium tail sgs
        elif rem > 4:
            kn = (rem + 1) // 2  # two small tail sgs
        else:
            kn = rem
        sg_list.append((k0, kn))
        k0 += kn

    nch_all, c0_all, mms_all, spans_all, nidx_all = [], [], [], [], []
    cpos = 0
    for (k0, kn) in sg_list:
        nch_sg = []
        c0_sg = []
        nidx_sg = []
        touches = []  # (slot, bank, j) -> sorted slot-major for PSUM groups
        covered = set()
        for b in range(NBANK):
            cum = np.zeros((ncores, kn + 1), np.int64)
            np.cumsum(counts[:, k0 : k0 + kn, b], axis=1, out=cum[:, 1:])
            maxn = int(max(cum[c, kn] for c in range(ncores)))
            nidx = -(-maxn // 16) * 16  # descriptors charged = num_idxs
            nch = -(-nidx // P)
            nch_sg.append(nch)
            nidx_sg.append(nidx)
            c0_sg.append(cpos)
            cpos += nch
            for j in range(nch):
                lo, hi = j * P, (j + 1) * P
                slots = set()
                for c in range(ncores):
                    for ki in range(kn):
                        if cum[c, ki] < hi and cum[c, ki + 1] > lo:
                            slots.add(ki)
                for ki in sorted(slots):
                    touches.append((ki, b, j))
                    covered.add(ki)
        # slot-major order: each slot's PSUM accumulation group closes
        # before the next one opens in the same PSUM bank
        entries = [(b, j, ki) for (ki, b, j) in sorted(touches)]
        # per-chunk slot span (for the wide one-hot build)
        spans = {}
        for (ki, b, j) in touches:
            lo, hi = spans.get((b, j), (ki, ki))
            spans[(b, j)] = (min(lo, ki), max(hi, ki))
        if sum(nch_sg) == 0:
            # fully empty supergroup: force one pad chunk in bank 0
            nch_sg[0] = 1
            nidx_sg[0] = 16
            for b in range(1, NBANK):
                c0_sg[b] = c0_sg[0] + 1
            cpos += 1
        # zero-coverage slots get one all-zero matmul on the sg's first
        # populated bank's chunk 0 (no core has a matching off there, by
        # construction)
        dummy_bank = next(b for b in range(NBANK) if nch_sg[b] > 0)
        for ki in range(kn):
            if ki not in covered:
                entries.append((dummy_bank, 0, ki))
        # start/stop per slot over entry order
        first, last = {}, {}
        for i, (b, j, ki) in enumerate(entries):
            if ki not in first:
                first[ki] = i
            last[ki] = i
        mms = [(b, j, ki, i == first[ki], i == last[ki])
               for i, (b, j, ki) in enumerate(entries)]
        nch_all.append(nch_sg)
        c0_all.append(c0_sg)
        mms_all.append(mms)
        spans_all.append(spans)
        nidx_all.append(nidx_sg)
    # idx column layout: per (sg, bank) call, nidx/16 int16 columns
    icol_all = []
    icol = 0
    for si in range(len(sg_list)):
        icol_sg = []
        for b in range(NBANK):
            icol_sg.append(icol)
            icol += nidx_all[si][b] // 16
        icol_all.append(icol_sg)
    return dict(sg_list=sg_list, nch=nch_all, c0=c0_all, mms=mms_all,
                spans=spans_all, nidx=nidx_all, icol=icol_all,
                n_chunks=cpos, ncols=icol)


def _fill_core_graph(plan, sched, core, cfg: Config):
    """Build IDX16 (wrapped), OFF and NSE arrays for one core, one graph."""
    n_chunks = sched["n_chunks"]
    idx = np.zeros((n_chunks, P), np.int16)
    off = np.full((n_chunks, P), OFF_PAD, np.float32)
    core_tiles = plan["core_tiles"]
    tile_edges = plan["tile_edges"]
    for si, (k0, kn) in enumerate(sched["sg_list"]):
        for b in range(NBANK):
            nch = sched["nch"][si][b]
            if nch == 0:
                continue
            c0 = sched["c0"][si][b]
            es = np.zeros(nch * P, np.int64)
            eo = np.full(nch * P, OFF_PAD, np.float32)
            pos = 0
            for ki in range(kn):
                t = core_tiles[core, k0 + ki]
                if t < 0:
                    continue
                s_rows, s_off = tile_edges[t][b]
                nb = len(s_rows)
                es[pos : pos + nb] = s_rows
                eo[pos : pos + nb] = ki * P + s_off
                pos += nb
            idx[c0 : c0 + nch] = es.reshape(nch, P)
            off[c0 : c0 + nch] = eo.reshape(nch, P)
    # wrap: flat slot i (within a call's first num_idxs slots) ->
    # [i%16, i//16], replicated to 128 partitions. Only num_idxs (<=
    # nch*128, 16-aligned) indices are shipped/charged per call; the
    # remaining tail of the last chunk is never gathered and its off
    # stays OFF_PAD.
    idx_w = np.zeros((P, sched["ncols"]), np.int16)
    for si in range(len(sched["sg_list"])):
        for b in range(NBANK):
            nidx = sched["nidx"][si][b]
            if nidx == 0:
                continue
            c0 = sched["c0"][si][b]
            icol = sched["icol"][si][b]
            flat = idx[c0 : c0 + sched["nch"][si][b]].reshape(-1)[:nidx]
            blk = flat.reshape(-1, 16).T  # [16, nidx/16]
            idx_w[:, icol : icol + nidx // 16] = np.tile(blk, (8, 1))
    return idx_w, off.T.copy()  # [P, n_chunks]


def preprocess(feats, W, b, prelu_a, src_pos, dst_pos, src_neg, dst_neg,
               cfg: Config):
    n, ncores = cfg.n_nodes, cfg.n_cores
    feats = np.asarray(feats, np.float32)
    W = np.asarray(W, np.float32)
    b = np.asarray(b, np.float32)
    prelu_a = np.asarray(prelu_a, np.float32)

    plans, scheds, featsrs, nds = [], [], [], []
    for src, dst in ((src_pos, dst_pos), (src_neg, dst_neg)):
        src = np.asarray(src, np.int64)
        dst = np.asarray(dst, np.int64)
        deg_out = np.bincount(src, minlength=n).astype(np.float32)
        deg_in = np.bincount(dst, minlength=n).astype(np.float32)
        ns = np.where(deg_out > 0, 1.0 / np.sqrt(np.maximum(deg_out, 1.0)),
                      0.0).astype(np.float32)
        nd = np.where(deg_in > 0, 1.0 / np.sqrt(np.maximum(deg_in, 1.0)),
                      0.0).astype(np.float32)
        # ns[src] folded into a per-graph bf16 copy of feats (gather cost
        # is unchanged); nd[dst] applied in f32 by the PReLU scale.
        fr = np.zeros((cfg.n_pad, D), BF16)
        fr[:n] = (feats * ns[:, None]).astype(BF16)
        featsrs.append(fr)
        nd_pad = np.zeros(cfg.n_pad, np.float32)
        nd_pad[:n] = nd
        nds.append(nd_pad)
        plan = _plan_graph(src, dst, cfg)
        plans.append(plan)
        scheds.append(_schedule_graph(plan, cfg))

    tcore = cfg.t_core
    nd_arrs = []
    for core in range(ncores):
        nda = np.zeros((P, 2 * tcore), np.float32)
        for g in range(2):
            ct = plans[g]["core_tiles"][core]
            for k in range(tcore):
                t = int(ct[k])
                if t >= 0:
                    nda[:, g * tcore + k] = nds[g][t * P : (t + 1) * P]
        nd_arrs.append(nda)

    ramp = np.tile(np.arange(cfg.sg * P, dtype=np.int16), (P, 1))
    a_rep = np.full((P, 1), float(prelu_a.reshape(-1)[0]), np.float32)
    b_rep = np.tile(b.reshape(1, D), (P, 1)).astype(np.float32)

    in_maps = []
    for core in range(ncores):
        iw_p, off_p = _fill_core_graph(plans[0], scheds[0], core, cfg)
        iw_n, off_n = _fill_core_graph(plans[1], scheds[1], core, cfg)
        in_maps.append({
            "featsr_p": featsrs[0],
            "featsr_n": featsrs[1],
            "w_in": W,
            "a_rep": a_rep,
            "b_rep": b_rep,
            "idx_in": np.concatenate([iw_p, iw_n], axis=1),
            "off_in": np.concatenate([off_p, off_n], axis=1),
            "nd_in": nd_arrs[core],
            "ramp_in": ramp,
        })
    meta = {
        "scheds": scheds,
        "use_bias": bool(np.any(b != 0.0)),
    }
    return in_maps, plans, meta


# --------------------------------------------------------------------------
# Device kernel builder
# --------------------------------------------------------------------------
def build_kernel(nc, tc, cfg: Config, meta):
    from contextlib import ExitStack

    import concourse.mybir as mybir

    f32 = mybir.dt.float32
    bf16 = mybir.dt.bfloat16
    i16 = mybir.dt.int16
    Alu = mybir.AluOpType
    Act = mybir.ActivationFunctionType

    npad = cfg.n_pad
    scheds = meta["scheds"]
    use_bias = meta["use_bias"]
    n_chunks = [scheds[g]["n_chunks"] for g in range(2)]
    ncols = [scheds[g]["ncols"] for g in range(2)]
    n_sg = len(scheds[0]["sg_list"])
    assert len(scheds[1]["sg_list"]) == n_sg

    featsr_p = nc.dram_tensor("featsr_p", [npad, D], bf16,
                              kind="ExternalInput").ap()
    featsr_n = nc.dram_tensor("featsr_n", [npad, D], bf16,
                              kind="ExternalInput").ap()
    featsr_g = [featsr_p, featsr_n]
    w_in = nc.dram_tensor("w_in", [P, D], f32, kind="ExternalInput").ap()
    a_rep = nc.dram_tensor("a_rep", [P, 1], f32, kind="ExternalInput").ap()
    b_rep = nc.dram_tensor("b_rep", [P, D], f32, kind="ExternalInput").ap()
    idx_in = nc.dram_tensor("idx_in", [P, sum(ncols)], i16,
                            kind="ExternalInput").ap()
    off_in = nc.dram_tensor("off_in", [P, sum(n_chunks)], f32,
                            kind="ExternalInput").ap()
    nd_in = nc.dram_tensor("nd_in", [P, 2 * cfg.t_core], f32,
                           kind="ExternalInput").ap()
    ramp_in = nc.dram_tensor("ramp_in", [P, cfg.sg * P], i16,
                             kind="ExternalInput").ap()
    out = nc.dram_tensor("out", [2, n_sg, P, cfg.sg * D], bf16,
                         kind="ExternalOutput").ap()

    with ExitStack() as ctx:
        const = ctx.enter_context(tc.tile_pool(name="const", bufs=1))
        gpool = ctx.enter_context(tc.tile_pool(name="gpool", bufs=cfg.gbufs))
        ipool = ctx.enter_context(tc.tile_pool(name="ipool", bufs=cfg.ipbufs))
        ohpool = ctx.enter_context(tc.tile_pool(name="ohpool", bufs=24))
        aggpool = ctx.enter_context(tc.tile_pool(name="aggpool", bufs=4))
        tpool = ctx.enter_context(tc.tile_pool(name="tpool", bufs=4))
        spool = ctx.enter_context(tc.tile_pool(name="spool", bufs=3))
        ppool = ctx.enter_context(tc.tile_pool(name="ppool", bufs=cfg.ppbufs,
                                               space="PSUM"))
        hpool = ctx.enter_context(tc.tile_pool(name="hpool", bufs=cfg.hpbufs,
                                               space="PSUM"))

        # ---- constants ----
        w_sb = const.tile([P, D], bf16)
        nc.gpsimd.dma_start(out=w_sb[:], in_=w_in)  # f32 -> bf16 cast DMA
        ramp_sb = const.tile([P, cfg.sg * P], i16)
        nc.gpsimd.dma_start(out=ramp_sb[:], in_=ramp_in)
        a_sb = const.tile([P, 1], f32)
        nc.gpsimd.dma_start(out=a_sb[:], in_=a_rep)
        off_sb = const.tile([P, sum(n_chunks)], f32)
        nc.gpsimd.dma_start(out=off_sb[:], in_=off_in)
        nd_sb = const.tile([P, 2 * cfg.t_core], f32)
        nc.gpsimd.dma_start(out=nd_sb[:], in_=nd_in)
        if use_bias:
            b_sb = const.tile([P, D], f32)
            nc.sync.dma_start(out=b_sb[:], in_=b_rep)

        max_sgc = max(sum(scheds[g]["nch"][si]) for g in range(2)
                      for si in range(n_sg))

        # ---- gather + weighted one-hot segment-sum + @W + prelu ----
        col_base = [0, ncols[0]]          # idx column offset per graph
        chk_base = [0, n_chunks[0]]       # off column offset per graph
        # interleave the two graphs' supergroups so one graph's gathers fill
        # DMA while the other's PSUM chain drains
        jobs = []
        for si in range(n_sg):
            for g in range(2):
                jobs.append((g, si))
        ecnt = 0
        for (g, si) in jobs:
            sch = scheds[g]
            (k0, kn) = sch["sg_list"][si]
            nchs = sch["nch"][si]
            c0s = sch["c0"][si]
            c0_sg = min(c0s[b] for b in range(NBANK) if nchs[b] > 0)
            sg_chunks = sum(nchs)
            nidxs = sch["nidx"][si]
            icols = sch["icol"][si]
            icol_sg = icols[0]
            icol_w = sum(nidxs) // 16
            gt = gpool.tile([P, max_sgc, D], bf16, tag="gather")
            it = ipool.tile([P, icol_w], i16, tag="gidx")
            nc.sync.dma_start(
                out=it[:],
                in_=idx_in[:, col_base[g] + icol_sg :
                           col_base[g] + icol_sg + icol_w])
            ot = off_sb[:, chk_base[g] + c0_sg :
                        chk_base[g] + c0_sg + sg_chunks]
            for b in range(NBANK):
                nch = nchs[b]
                if nch == 0:
                    continue
                lo = c0s[b] - c0_sg
                ilo = icols[b] - icol_sg
                nidx = nidxs[b]
                bank_rows = min(cfg.bank_rows, npad - b * cfg.bank_rows)
                nc.gpsimd.dma_gather(
                    out_ap=gt[:, lo : lo + nch, :],
                    in_ap=featsr_g[g][b * cfg.bank_rows :
                                      b * cfg.bank_rows + bank_rows, :],
                    idxs_ap=it[:, ilo : ilo + nidx // 16],
                    num_idxs=nidx, num_idxs_reg=nidx,
                    elem_size=D, single_packet=False)
            nquad = (kn + 3) // 4
            psTs = [ppool.tile([P, 4 * D], f32, tag="psT", name="psT")
                    for _ in range(nquad)]
            spans = sch["spans"][si]
            oh_cache = {}
            for (b, j, ki, st, sp) in sch["mms"][si]:
                lo = c0s[b] - c0_sg + j
                pv = min(P, nidxs[b] - j * P)  # valid rows in this chunk
                klo, khi = spans.get((b, j), (ki, ki))
                if ki < klo or ki > khi:
                    # dummy zero matmul for an uncovered slot: one-off
                    # narrow one-hot (off values there never match ki)
                    oh = ohpool.tile([P, D], bf16, tag="ohw", name="ohw")
                    nc.vector.tensor_scalar(
                        out=oh[:], in0=ramp_sb[:, ki * P : (ki + 1) * P],
                        scalar1=ot[:, lo : lo + 1],
                        scalar2=None, op0=Alu.is_equal)
                    rhs = oh[:pv, :]
                else:
                    if (b, j) not in oh_cache:
                        span = khi - klo + 1
                        ohw = ohpool.tile([P, span * D], bf16, tag="ohw",
                                          name="ohw")
                        eng = nc.vector
                        if cfg.oh_gpsimd_mod and (
                                ecnt % cfg.oh_gpsimd_mod == 0):
                            eng = nc.gpsimd
                        ecnt += 1
                        eng.tensor_scalar(
                            out=ohw[:],
                            in0=ramp_sb[:, klo * P : (khi + 1) * P],
                            scalar1=ot[:, lo : lo + 1],
                            scalar2=None, op0=Alu.is_equal)
                        oh_cache[(b, j)] = ohw
                    ohw = oh_cache[(b, j)]
                    rhs = ohw[:pv, (ki - klo) * D : (ki - klo + 1) * D]
                q, r = divmod(ki, 4)
                nc.tensor.matmul(
                    out=psTs[q][:, r * D : (r + 1) * D],
                    lhsT=gt[:pv, lo, :], rhs=rhs, start=st, stop=sp)
            stg = spool.tile([P, kn * D], bf16, tag="stg")
            for q in range(nquad):
                kq = min(4, kn - 4 * q)
                aggsb = aggpool.tile([P, 4 * D], bf16, tag="aggsb")
                nc.scalar.activation(out=aggsb[:, : kq * D],
                                     in_=psTs[q][:, : kq * D],
                                     func=Act.Copy)
                hps = hpool.tile([P, 4 * D], f32)
                for r in range(kq):
                    nc.tensor.matmul(out=hps[:, r * D : (r + 1) * D],
                                     lhsT=aggsb[:, r * D : (r + 1) * D],
                                     rhs=w_sb[:], start=True, stop=True)
                for r in range(kq):
                    kslot = g * cfg.t_core + k0 + 4 * q + r
                    ndc = nd_sb[:, kslot : kslot + 1]
                    hr = hps[:, r * D : (r + 1) * D]
                    sr = stg[:, (4 * q + r) * D : (4 * q + r + 1) * D]
                    if cfg.act_prelu and not use_bias:
                        nc.scalar.activation(
                            out=sr, in_=hr, func=Act.Prelu,
                            scale=ndc, alpha=a_sb[:, :1])
                        continue
                    hn = tpool.tile([P, D], f32, tag="hn")
                    if use_bias:
                        nc.vector.tensor_scalar(
                            out=hn[:], in0=hr, scalar1=ndc,
                            scalar2=None, op0=Alu.mult)
                        nc.vector.tensor_tensor(out=hn[:], in0=hn[:],
                                                in1=b_sb[:, :D], op=Alu.add)
                    else:
                        nc.vector.tensor_scalar(
                            out=hn[:], in0=hr, scalar1=ndc,
                            scalar2=None, op0=Alu.mult)
                    neg = tpool.tile([P, D], f32, tag="neg")
                    nc.vector.tensor_scalar(
                        out=neg[:], in0=hn[:], scalar1=0.0,
                        scalar2=a_sb[:, :1], op0=Alu.min, op1=Alu.mult)
                    pos = tpool.tile([P, D], f32, tag="pos")
                    nc.vector.tensor_scalar(
                        out=pos[:], in0=hn[:], scalar1=0.0,
                        scalar2=None, op0=Alu.max)
                    nc.vector.tensor_tensor(out=sr, in0=neg[:],
                                            in1=pos[:], op=Alu.add)
            nc.sync.dma_start(out=out[g, si, :, : kn * D], in_=stg[:])
    return out


# --------------------------------------------------------------------------
# Driver
# --------------------------------------------------------------------------
def _build_program(cfg: Config, meta):
    import concourse.bacc as bacc
    import concourse.tile as tile

    nc = bacc.Bacc("TRN2", target_bir_lowering=False, debug=False,
                   enable_asserts=False, num_devices=cfg.n_cores)
    with tile.TileContext(nc) as tc:
        build_kernel(nc, tc, cfg, meta)
    nc.compile()
    return nc


def _unscramble(results, plans, meta, cfg: Config):
    n = cfg.n_nodes
    full = np.zeros((2, n, D), np.float32)
    for g in range(2):
        sched = meta["scheds"][g]
        ct_all = plans[g]["core_tiles"]
        for core in range(cfg.n_cores):
            oc = np.asarray(results[core]["out"], dtype=np.float32)
            for si, (k0, kn) in enumerate(sched["sg_list"]):
                for ki in range(kn):
                    t = int(ct_all[core, k0 + ki])
                    if t < 0:
                        continue
                    r0 = t * P
                    r1 = min(r0 + P, n)
                    full[g, r0:r1] = oc[g, si, : r1 - r0,
                                        ki * D : (ki + 1) * D]
    return full


_PROGRAM_CACHE = {}


def _sched_key(sched):
    return (tuple(tuple(x) for x in sched["nch"]),
            tuple(mm for sgm in sched["mms"] for mm in sgm))


def run(inputs, cfg: Config, trace=False):
    from concourse.bass_utils import run_bass_kernel_spmd

    in_maps, plans, meta = preprocess(
        inputs["feats"], inputs["W"], inputs["b"], inputs["prelu_a"],
        inputs["src_pos"], inputs["dst_pos"],
        inputs["src_neg"], inputs["dst_neg"], cfg)

    key = (cfg.n_nodes, cfg.n_cores, cfg.sg,
           cfg.act_prelu, cfg.oh_gpsimd_mod, cfg.gbufs,
           _sched_key(meta["scheds"][0]), _sched_key(meta["scheds"][1]),
           meta["use_bias"])
    nc = _PROGRAM_CACHE.get(key)
    if nc is None:
        nc = _build_program(cfg, meta)
        _PROGRAM_CACHE[key] = nc

    kwargs = {}
    if trace:
        kwargs = dict(trace=True, tmpdir=tempfile.mkdtemp(prefix="bgc_trace_"))
    res = run_bass_kernel_spmd(nc, in_maps, core_ids=list(range(cfg.n_cores)),
                               **kwargs)
    full = _unscramble(res.results, plans, meta, cfg)
    return full, res


def kernel(**inputs) -> np.ndarray:
    cfg = Config()
    full, _ = run(inputs, cfg)
    return full
